# revision 39
# baseline (speedup 1.0000x reference)
"""Trainium2 Bass kernel for nn_Estor_45595372814585 (span transformer block).

Sharding: batch b -> NeuronCore b (8 batches, 8 cores), no collectives.
QKV / out-proj matmuls in fp8e4 with DoubleRow perf mode (2 k-chunks per
instruction); scores/softmax/AV in bf16; FFN selectable bf16 or fp8
(KFFN env). Residuals/LayerNorm in fp32.

Scaling convention: activations entering fp8 matmuls are pre-scaled by
AS, weights by WS (Q weights additionally by SCALE). PSUM therefore
holds AS*WS*(x@w); copy-outs divide back.
"""
import sys, os
sys.path.insert(0, '/opt/trn_rl_repo')
KFFN = os.environ.get("KFFN", "mix2")     # "bf16" | "fp8" | "mix1" | "mix2"
KPHASE = int(os.environ.get("KPHASE", "99"))
import numpy as np
import ml_dtypes

from concourse import bass, bacc, mybir, tile
from concourse.bass_utils import run_bass_kernel_spmd
from concourse.masks import make_identity

F32 = mybir.dt.float32
BF16 = mybir.dt.bfloat16
FP8 = mybir.dt.float8e4
I32 = mybir.dt.int32
AF = mybir.ActivationFunctionType
OP = mybir.AluOpType
DR = mybir.MatmulPerfMode.DoubleRow

B, S, H, NH, NT, NL, I = 8, 2048, 768, 12, 32, 9, 3072
SPAN, NSP, DH = 64, 32, 64
TAG_RATE = 2.0
EPS = 1e-12
NC_H = H // 128          # 6 hidden chunks
NC_I = I // 128          # 24 ffn chunks
NTOK = NSP * SPAN        # 2048 gathered tokens
NT16 = NTOK // 128       # 16 token tiles
SCALE = 1.0 / (DH ** 0.5)
AS = 8.0                 # activation fp8 pre-scale
WS = 32.0                # weight fp8 pre-scale
SKEY2 = SPAN + 1         # layer-2 keys per span (64 tokens + tag)

NBLK = 4                 # span blocks for the fused out-projections
SPB = NSP // NBLK        # 8 spans / block
TBLK = SPB * SPAN        # 512 tokens / block

_CACHE = {}


def _bcast_vec(nc, ps_pool, sb_pool, ones_f32, src_dram, n, tag, row_pool=None):
    """Broadcast a [1, n] f32 DRAM vector to a [128, n] f32 SBUF tile via PE."""
    row = (row_pool or sb_pool).tile([1, n], F32, tag="rowtmp", bufs=2, name="rowtmp")
    nc.sync.dma_start(row[:], src_dram[:])
    out = sb_pool.tile([128, n], F32, tag=tag + "_bc", bufs=1)
    for o in range(0, n, 512):
        w = min(512, n - o)
        p = ps_pool.tile([128, 512], F32, tag="bcps")
        nc.tensor.matmul(p[:, :w], ones_f32[:, :], row[:, o:o + w], start=True, stop=True)
        nc.vector.tensor_copy(out[:, o:o + w], p[:, :w])
    return out


def _ln_apply(nc, sb, x_tile, cols, scratch_tag, g_bc, b_bc, trivial, eps_t, out_ap):
    """LayerNorm over free-dim H on x_tile[:, cols] (f32, [128, 768]);
    writes out_ap (f32)."""
    mean = sb.tile([128, 1], F32, tag=scratch_tag + "_m")
    negm = sb.tile([128, 1], F32, tag=scratch_tag + "_nm")
    ss = sb.tile([128, 1], F32, tag=scratch_tag + "_ss")
    sq = sb.tile([128, H], F32, tag=scratch_tag + "_sq")
    rstd = sb.tile([128, 1], F32, tag=scratch_tag + "_rs")
    xin = x_tile[:, cols]
    nc.vector.reduce_sum(out=mean[:], in_=xin, axis=mybir.AxisListType.X)
    nc.vector.tensor_scalar_mul(negm[:], mean[:], -1.0 / H)
    sd = sb.tile([128, 1], F32, tag=scratch_tag + "_sd")
    nc.scalar.activation(sq[:], xin, AF.Square, bias=negm[:, :1], accum_out=ss[:])
    nc.scalar.activation(sd[:], ss[:], AF.Sqrt, bias=eps_t[:, :1], scale=1.0 / H)
    nc.vector.reciprocal_approx_fast(out=rstd[:], in_=sd[:])
    if trivial:
        nc.gpsimd.tensor_scalar(out=out_ap, in0=xin, scalar1=negm[:, :1],
                                scalar2=rstd[:, :1], op0=OP.add, op1=OP.mult)
    else:
        nc.gpsimd.tensor_scalar(out=sq[:], in0=xin, scalar1=negm[:, :1],
                                scalar2=rstd[:, :1], op0=OP.add, op1=OP.mult)
        nc.vector.tensor_tensor(out=sq[:], in0=sq[:], in1=g_bc[:], op=OP.mult)
        nc.vector.tensor_tensor(out=out_ap, in0=sq[:], in1=b_bc[:], op=OP.add)


def _ln_fast(nc, sb, xin, tag, eps_t, out_ap, apply_eng=None):
    """Trivial LayerNorm (g=1, b=0) with independent mean / E[x^2]
    reductions so neither engine waits on the other: var = E[x^2]-m^2."""
    ssum = sb.tile([128, 1], F32, tag=tag + "_s")
    negm = sb.tile([128, 1], F32, tag=tag + "_nm")
    msq = sb.tile([128, 1], F32, tag=tag + "_m2")
    ss = sb.tile([128, 1], F32, tag=tag + "_ss")
    var = sb.tile([128, 1], F32, tag=tag + "_v")
    sd = sb.tile([128, 1], F32, tag=tag + "_sd")
    rstd = sb.tile([128, 1], F32, tag=tag + "_rs")
    sq = sb.tile([128, H], BF16, tag=tag + "_sq")
    nc.vector.reduce_sum(out=ssum[:], in_=xin, axis=mybir.AxisListType.X)
    nc.scalar.activation(sq[:], xin, AF.Square, accum_out=ss[:])
    nc.vector.tensor_scalar_mul(negm[:], ssum[:], -1.0 / H)
    nc.vector.tensor_tensor(out=msq[:], in0=negm[:], in1=negm[:], op=OP.mult)
    nc.vector.scalar_tensor_tensor(out=var[:], in0=ss[:], scalar=1.0 / H,
                                   op0=OP.mult, in1=msq[:], op1=OP.subtract)
    nc.scalar.activation(sd[:], var[:], AF.Sqrt, bias=eps_t[:, :1])
    nc.vector.reciprocal_approx_fast(out=rstd[:], in_=sd[:])
    (apply_eng or nc.gpsimd).tensor_scalar(
        out=out_ap, in0=xin, scalar1=negm[:, :1],
        scalar2=rstd[:, :1], op0=OP.add, op1=OP.mult)


def build(kranges, an_trivial, fn_trivial, bias0=True, identg=False, ffn_mode=KFFN):
    nc = _build_ir(kranges, an_trivial, fn_trivial, bias0, identg, ffn_mode)
    nc.compile()
    return nc


def _build_ir(kranges, an_trivial, fn_trivial, bias0, identg, ffn_mode):
    nc = bacc.Bacc("TRN2", target_bir_lowering=False, debug=False, num_devices=8)
    f1 = ffn_mode in ("fp8", "mix1")   # stage-1 (W1) matmul in fp8
    f2 = ffn_mode in ("fp8", "mix2")   # stage-2 (W2) matmul in fp8
    NPAIR = sum(len(r) for r in kranges)

    # ---- DRAM I/O ----
    we = nc.dram_tensor("we", [S, H], F32, kind="ExternalInput")
    web = nc.dram_tensor("web", [S, H], BF16, kind="ExternalInput")
    posi = nc.dram_tensor("posi", [128, NT16], I32, kind="ExternalInput")
    cosg = nc.dram_tensor("cosg", [NTOK, H // 2], BF16, kind="ExternalInput")
    sing = nc.dram_tensor("sing", [NTOK, H // 2], BF16, kind="ExternalInput")
    mmatc = nc.dram_tensor("mmatc", [max(NPAIR, 1) * 128, 128], BF16,
                           kind="ExternalInput")
    tgKd = nc.dram_tensor("tgKd", [H, NSP], BF16, kind="ExternalInput")
    tgVd = nc.dram_tensor("tgVd", [1, NSP * H], BF16, kind="ExternalInput")
    # weights pre-layouted on host to [128, ...] so every stream DMA is a
    # contiguous >=512B-per-descriptor copy
    wqkv1q = nc.dram_tensor("wq1p", [128, 18 * H], FP8, kind="ExternalInput")
    wqkv2q = nc.dram_tensor("wq2p", [128, 18 * H], FP8, kind="ExternalInput")
    woq1 = nc.dram_tensor("wo1p", [128, NC_H * H], FP8, kind="ExternalInput")
    woq2 = nc.dram_tensor("wo2p", [128, NC_H * H], FP8, kind="ExternalInput")
    W1DT = FP8 if f1 else BF16
    W2DT = FP8 if f2 else BF16
    H1DT = FP8 if f2 else BF16
    w1d = nc.dram_tensor("w1pp", [128, NC_I * NC_H * 128], W1DT,
                         kind="ExternalInput")
    w2d = nc.dram_tensor("w2pp", [128, NC_I * H], W2DT,
                         kind="ExternalInput")
    woutT = nc.dram_tensor("woutT", [H, NL], BF16, kind="ExternalInput")
    # bias columns ([128, n_chunks] f32, chunk c in col c)
    bq1c = nc.dram_tensor("bq1c", [128, NC_H], F32, kind="ExternalInput")  # SCALE*bq1
    bk1c = nc.dram_tensor("bk1c", [128, NC_H], F32, kind="ExternalInput")  # bk1
    bq2c = nc.dram_tensor("bq2c", [128, NC_H], F32, kind="ExternalInput")  # SCALE*bq2
    bk2c = nc.dram_tensor("bk2c", [128, NC_H], F32, kind="ExternalInput")  # bk2
    vec_bo1r = nc.dram_tensor("vec_bo1r", [1, H], F32, kind="ExternalInput")  # AS*WS*bo1
    b1c = nc.dram_tensor("b1c", [128, NC_I], F32, kind="ExternalInput")    # (AS*)b1
    # bias rows
    vec_bv1 = nc.dram_tensor("vec_bv1", [1, H], F32, kind="ExternalInput")
    vec_bv2 = nc.dram_tensor("vec_bv2", [1, H], F32, kind="ExternalInput")
    vec_bo2 = nc.dram_tensor("vec_bo2", [1, H], F32, kind="ExternalInput")
    vec_b2 = nc.dram_tensor("vec_b2", [1, H], F32, kind="ExternalInput")   # (AS*WS*)b2
    vec_ang = nc.dram_tensor("vec_ang", [1, H], F32, kind="ExternalInput")
    vec_anb = nc.dram_tensor("vec_anb", [1, H], F32, kind="ExternalInput")
    vec_fng = nc.dram_tensor("vec_fng", [1, H], F32, kind="ExternalInput")
    vec_fnb = nc.dram_tensor("vec_fnb", [1, H], F32, kind="ExternalInput")
    vec_bout = nc.dram_tensor("vec_bout", [1, NL], F32, kind="ExternalInput")
    out_d = nc.dram_tensor("out", [S, NL], F32, kind="ExternalOutput")

    QCP = 1.0 / (AS * WS)   # generic fp8 psum descale

    with tile.TileContext(nc) as tc:
        with tc.tile_pool(name="const", bufs=1) as csb:
            ones_f32 = csb.tile([1, 128], F32, tag="ones_f32")
            nc.vector.memset(ones_f32[:], 1.0)
            ones_col_bf = csb.tile([128, 1], BF16, tag="ones_col_bf")
            nc.vector.memset(ones_col_bf[:], 1.0)
            ones_row_bf = csb.tile([1, 128], BF16, tag="ones_row_bf")
            nc.vector.memset(ones_row_bf[:], 1.0)
            as12 = csb.tile([NH, 64], BF16, tag="as12")
            nc.vector.memset(as12[:], AS)
            ones_tb = None
            if not bias0:
                ones_tb = csb.tile([1, TBLK], BF16, tag="ones_tb")
                nc.vector.memset(ones_tb[:], 1.0)
            ident = csb.tile([128, 128], F32, tag="ident")
            make_identity(nc, ident[:])
            ident_bf = csb.tile([128, 128], BF16, tag="ident_bf")
            nc.vector.tensor_copy(ident_bf[:], ident[:])
            # asel[h, j*64+f] = AS if h == j else 0 — selector for broadcasting
            # one head-row of rec_h across 64 feature partitions
            asel = csb.tile([NH, NH * 64], BF16, tag="asel")
            for j in range(NH):
                nc.vector.tensor_scalar(
                    out=asel[:, j * 64:(j + 1) * 64], in0=as12[:, :],
                    scalar1=ident[0:NH, j:j + 1], scalar2=None, op0=OP.mult)
            eps_t = csb.tile([128, 1], F32, tag="eps")
            nc.vector.memset(eps_t[:], EPS)
            bq1 = bk1 = bq2 = bk2 = b1t = bo1row = None
            if not bias0:
                bq1 = csb.tile([128, NC_H], F32, tag="bq1")
                nc.sync.dma_start(bq1[:], bq1c[:])
                bk1 = csb.tile([128, NC_H], F32, tag="bk1")
                nc.sync.dma_start(bk1[:], bk1c[:])
                bq2 = csb.tile([128, NC_H], F32, tag="bq2")
                nc.sync.dma_start(bq2[:], bq2c[:])
                bk2 = csb.tile([128, NC_H], F32, tag="bk2")
                nc.sync.dma_start(bk2[:], bk2c[:])
                b1t = csb.tile([128, NC_I], F32, tag="b1t")
                nc.sync.dma_start(b1t[:], b1c[:])
                bo1row_f = csb.tile([1, H], F32, tag="bo1row_f")
                nc.sync.dma_start(bo1row_f[:], vec_bo1r[:])
                bo1row = csb.tile([1, H], BF16, tag="bo1row")
                nc.vector.tensor_copy(bo1row[:], bo1row_f[:])

            with tc.tile_pool(name="cps", bufs=1, space="PSUM") as cps, \
                 tc.tile_pool(name="crow", bufs=1) as crow:
                bv1_bc = bv2_bc = bo2_bc = bout_bc = None
                if not bias0:
                    bv1_bc = _bcast_vec(nc, cps, csb, ones_f32, vec_bv1, H, "bv1",
                                        row_pool=crow)
                    bv2_bc = _bcast_vec(nc, cps, csb, ones_f32, vec_bv2, H, "bv2",
                                        row_pool=crow)
                    bo2_bc = _bcast_vec(nc, cps, csb, ones_f32, vec_bo2, H, "bo2",
                                        row_pool=crow)
                    bout_bc = _bcast_vec(nc, cps, csb, ones_f32, vec_bout, NL, "bout",
                                         row_pool=crow)
                ang_bc = anb_bc = fng_bc = fnb_bc = None
                if not an_trivial:
                    ang_bc = _bcast_vec(nc, cps, csb, ones_f32, vec_ang, H, "ang", row_pool=crow)
                    anb_bc = _bcast_vec(nc, cps, csb, ones_f32, vec_anb, H, "anb", row_pool=crow)
                if not fn_trivial:
                    fng_bc = _bcast_vec(nc, cps, csb, ones_f32, vec_fng, H, "fng", row_pool=crow)
                    fnb_bc = _bcast_vec(nc, cps, csb, ones_f32, vec_fnb, H, "fnb", row_pool=crow)
                b2row = None
                if not bias0:
                    b2row_f = crow.tile([1, H], F32, tag="b2row_f")
                    nc.sync.dma_start(b2row_f[:], vec_b2[:])
                    b2row = csb.tile([1, H], BF16, tag="b2row")
                    nc.vector.tensor_copy(b2row[:], b2row_f[:])

            # ---------------- shared big SBUF slots ----------------
            with tc.tile_pool(name="pbig", bufs=1) as pbig:
                def big24(name):
                    return pbig.tile([128, NC_H * NTOK], BF16, tag="p24", bufs=1,
                                     name=name)

                def big12(name):
                    return pbig.tile([128, NC_H * NTOK], FP8, tag="p12", bufs=2,
                                     name=name)

                # ========= Phase 1: gather + rope(->AS-scaled) + transpose =========
                Xt8 = big12("Xt8")            # AS * rope(x), fp8, T-layout
                xbf_cm = tc.tile_pool(name="xbfp", bufs=1, side="right")
                xbfp = xbf_cm.__enter__()
                # AS * (rope(x) + bo1), bf16, T-layout; dies after attention 1
                Xbf = xbfp.tile([128, NC_H * NTOK], BF16, tag="xbf", bufs=1,
                                name="Xbf")
                with tc.tile_pool(name="p1sb", bufs=2) as p1, \
                     tc.tile_pool(name="p1ps", bufs=2, space="PSUM") as p1ps:
                    if not identg:
                        idx_t = p1.tile([128, NT16], I32, tag="idx", bufs=1)
                        nc.sync.dma_start(idx_t[:], posi[:])
                    for t in range(NT16):
                        g = p1.tile([128, H], BF16, tag="g")
                        if identg:
                            nc.sync.dma_start(g[:], web[t * 128:(t + 1) * 128, :])
                        else:
                            nc.gpsimd.indirect_dma_start(
                                out=g[:], out_offset=None, in_=web[:],
                                in_offset=bass.IndirectOffsetOnAxis(
                                    ap=idx_t[:, t:t + 1], axis=0))
                        cos_t = p1.tile([128, H // 2], BF16, tag="cos")
                        sin_t = p1.tile([128, H // 2], BF16, tag="sin")
                        nc.sync.dma_start(cos_t[:], cosg[t * 128:(t + 1) * 128, :])
                        nc.sync.dma_start(sin_t[:], sing[t * 128:(t + 1) * 128, :])
                        # cos/sin tables carry the AS prescale (host-side)
                        ge = g[:, 0:H:2]
                        go = g[:, 1:H:2]
                        t0 = p1.tile([128, H // 2], BF16, tag="t0")
                        t1 = p1.tile([128, H // 2], BF16, tag="t1")
                        rp = p1.tile([128, H], BF16, tag="rp")
                        nc.vector.tensor_tensor(out=t0[:], in0=ge, in1=cos_t[:], op=OP.mult)
                        nc.gpsimd.tensor_tensor(out=t1[:], in0=go, in1=sin_t[:], op=OP.mult)
                        nc.vector.tensor_tensor(out=rp[:, 0:H:2], in0=t0[:], in1=t1[:],
                                                op=OP.subtract)
                        nc.gpsimd.tensor_tensor(out=t0[:], in0=ge, in1=sin_t[:], op=OP.mult)
                        nc.vector.tensor_tensor(out=t1[:], in0=go, in1=cos_t[:], op=OP.mult)
                        nc.gpsimd.tensor_tensor(out=rp[:, 1:H:2], in0=t0[:], in1=t1[:],
                                                op=OP.add)
                        trp = p1ps.tile([128, H], BF16, tag="trp")
                        for c in range(NC_H):
                            nc.tensor.transpose(trp[:, c * 128:(c + 1) * 128],
                                                rp[:, c * 128:(c + 1) * 128],
                                                ident_bf[:])
                        trpv = trp[:].rearrange("p (c n) -> p c n", c=NC_H)
                        x8v = Xt8[:].rearrange("p (c n) -> p c n", c=NC_H)
                        xbv = Xbf[:].rearrange("p (c n) -> p c n", c=NC_H)
                        nc.scalar.activation(
                            x8v[:, :, t * 128:(t + 1) * 128], trpv[:, :, :],
                            AF.Copy)
                        nc.vector.tensor_copy(
                            xbv[:, :, t * 128:(t + 1) * 128], trpv[:, :, :])

                # ============ attention layer ============
                def attention(lidx, Xin8, bq, bk, wqkvT, woT, bv_bc, X1q, tgKs, lp,
                              wpools, post_blk=None):
                    """lidx 1: self-attn over spans; returns nothing (writes X1q).
                    lidx 2: tag-augmented attn; returns res2 (natural layout).
                    lp: attention-local tile pool (Q/K/V die at return)."""
                    if lidx == 1:
                        g_qk, g_v, g_at, g_av, g_op = 2, 3, 4, 5, 6
                    else:
                        g_qk, g_v, g_at, g_av, g_op = 7, 7, 8, 8, 9
                    KTOT = SPAN if lidx == 1 else SKEY2
                    Qt = lp.tile([128, NC_H * NTOK], BF16, tag="qt", bufs=1)
                    if lidx == 1:
                        Kt = lp.tile([128, NC_H * NTOK], BF16, tag="kt", bufs=1)
                        ktv = None
                    else:
                        Kt = lp.tile([128, NC_H * NSP * SKEY2], BF16, tag="kt2",
                                     bufs=1)
                        ktv = Kt[:].rearrange("p (c s k) -> p c s k", c=NC_H, k=SKEY2)
                    Vn = lp.tile([SKEY2, NSP * H], BF16, tag="vn", bufs=1)
                    Xv = Xin8[:].rearrange("p (c n) -> p c n", c=NC_H)
                    wstr, wpool, awstr = wpools
                    with tc.tile_pool(name=f"qkvp{lidx}", bufs=3, space="PSUM") as qps, \
                         tc.tile_pool(name=f"qkvv{lidx}", bufs=2, space="PSUM") as vqps:
                        for qk, bcol in (((0, bq), (1, bk)) if KPHASE >= g_qk else ()):
                            for oc in range(NC_H):
                                wqo = wstr.tile([128, NC_H * 128], FP8, tag="wqo")
                                nc.sync.dma_start(
                                    wqo[:],
                                    wqkvT[:, (qk * NC_H + oc) * H:
                                          (qk * NC_H + oc + 1) * H])
                                wv_ = wqo[:].rearrange("p (c n) -> p c n", c=NC_H)
                                for tkc in range(4):
                                    ps = qps.tile([128, 512], F32, tag="qkps")
                                    for j in range(NC_H // 2):
                                        nc.tensor.matmul(
                                            ps[:],
                                            wv_[:, 2 * j:2 * j + 2, :],
                                            Xv[:, 2 * j:2 * j + 2,
                                               tkc * 512:(tkc + 1) * 512],
                                            start=(j == 0), stop=(j == NC_H // 2 - 1),
                                            perf_mode=DR)
                                    if qk == 0 or lidx == 1:
                                        dst = (Qt if qk == 0 else Kt)[
                                            :, oc * NTOK + tkc * 512:
                                               oc * NTOK + (tkc + 1) * 512]
                                    else:
                                        sp0 = tkc * 8
                                        dst = ktv[:, oc:oc + 1, sp0:sp0 + 8, 0:SPAN]
                                    use_dve = (oc % 2 == 1)
                                    if bias0 and use_dve:
                                        nc.vector.tensor_scalar_mul(dst, ps[:], QCP)
                                    elif bias0:
                                        nc.scalar.activation(dst, ps[:], AF.Copy,
                                                             scale=QCP)
                                    else:
                                        nc.scalar.activation(dst, ps[:], AF.Identity,
                                                             bias=bcol[:, oc:oc + 1],
                                                             scale=QCP)
                        if lidx == 2 and KPHASE >= g_v:
                            # tag K column
                            nc.gpsimd.tensor_copy(
                                ktv[:, :, :, SPAN:SKEY2],
                                tgKs[:].rearrange("p (c s) -> p c s", c=NC_H))
                        # V per span (tokens on psum partitions 0-63)
                        wv = wpool.tile([128, NC_H * H], FP8, tag="wv")
                        nc.sync.dma_start(wv[:], wqkvT[:, 12 * H: 18 * H])
                        wvv = wv[:].rearrange("p (c n) -> p c n", c=NC_H)
                        if lidx == 2 and KPHASE >= g_v:
                            # tag V row: slow single-partition DMA — run it on the
                            # Activation HWDGE queue so it overlaps the SP-queue
                            # weight streams, chunked per span block
                            for tb8 in range(8):
                                w8_ = (NSP // 8) * H
                                nc.sync.dma_start(
                                    Vn[SPAN:SKEY2, tb8 * w8_:(tb8 + 1) * w8_],
                                    tgVd[0:1, tb8 * w8_:(tb8 + 1) * w8_])
                        for sp in range(NSP if KPHASE >= g_v else 0):
                            ps = vqps.tile([64, H], F32, tag="vps")
                            for no in range(2):
                                nw = 512 if no == 0 else 256
                                for j in range(NC_H // 2):
                                    nc.tensor.matmul(
                                        ps[:, no * 512: no * 512 + nw],
                                        Xv[:, 2 * j:2 * j + 2,
                                           sp * SPAN:(sp + 1) * SPAN],
                                        wvv[:, 2 * j:2 * j + 2,
                                            no * 512:no * 512 + nw],
                                        start=(j == 0), stop=(j == NC_H // 2 - 1),
                                        perf_mode=DR)
                            if bias0:
                                if sp % 2 == 0:
                                    nc.scalar.activation(
                                        Vn[0:SPAN, sp * H:(sp + 1) * H], ps[:],
                                        AF.Copy, scale=QCP)
                                else:
                                    nc.vector.tensor_scalar_mul(
                                        Vn[0:SPAN, sp * H:(sp + 1) * H], ps[:], QCP)
                            else:
                                nc.vector.scalar_tensor_tensor(
                                    out=Vn[0:SPAN, sp * H:(sp + 1) * H],
                                    in0=ps[:], scalar=QCP, op0=OP.mult,
                                    in1=bv_bc[0:SPAN, :], op1=OP.add)

                    ret = big24("res2") if lidx == 2 else None
                    with tc.tile_pool(name=f"a{lidx}sb", bufs=3) as asb, \
                         tc.tile_pool(name=f"a{lidx}ps", bufs=1, space="PSUM") as sps, \
                         tc.tile_pool(name=f"a{lidx}dn", bufs=1, space="PSUM") as dps, \
                         tc.tile_pool(name=f"a{lidx}rt", bufs=1, space="PSUM") as rps, \
                         tc.tile_pool(name=f"a{lidx}av", bufs=2, space="PSUM") as avps, \
                         tc.tile_pool(name=f"a{lidx}op", bufs=1, space="PSUM") as opps:
                        # stream out-proj weights once (fp8: 4.5KB/part)
                        wos = awstr.tile([128, NC_H * H], FP8, tag="wos", bufs=1)
                        nc.sync.dma_start(wos[:], woT[:])
                        wosv = wos[:].rearrange("p (c n) -> p c n", c=NC_H)

                        # --- software-pipelined span loop: stage1 (scores+
                        # exp) runs 2 spans ahead of stage3 (recT+normalize)
                        # so the in-order PE queue never waits on the
                        # exp->den->reciprocal->broadcast chain ---
                        st = {}
                        aobs = {}

                        def stage1(s):
                            c0 = s * SPAN
                            scE = sps.tile([KTOT, 384], F32, tag="scE")
                            scO = sps.tile([KTOT, 384], F32, tag="scO")
                            for h in range(NH):
                                hb = (h % 2) * 64
                                dst = scE if h % 2 == 0 else scO
                                if lidx == 1:
                                    kap = Kt[hb:hb + 64,
                                             (h // 2) * NTOK + c0:
                                             (h // 2) * NTOK + c0 + SPAN]
                                else:
                                    kap = ktv[hb:hb + 64, h // 2:h // 2 + 1,
                                              s:s + 1, :]
                                nc.tensor.matmul(
                                    dst[:, (h // 2) * 64:(h // 2 + 1) * 64],
                                    kap,
                                    Qt[hb:hb + 64, (h // 2) * NTOK + c0:
                                       (h // 2) * NTOK + c0 + SPAN],
                                    start=True, stop=True)
                            pexpE = asb.tile([KTOT, 384], BF16, tag="pexpE")
                            pexpO = asb.tile([KTOT, 384], BF16, tag="pexpO")
                            nc.scalar.activation(pexpE[:], scE[:], AF.Exp)
                            nc.scalar.activation(pexpO[:], scO[:], AF.Exp)
                            st[s] = [(pexpE, pexpO)]

                        def stage2(s):
                            pexpE, pexpO = st[s][0]
                            # av_t packs: [:,0:384] attn@V; [0:64,384:396] den;
                            # [0:12,400:464] transposed reciprocal
                            av_t = avps.tile([128, 384], F32, tag="av", bufs=2)
                            den_t = dps.tile([64, NH], F32, tag="den_t")
                            for h in range(NH):
                                pex = pexpE if h % 2 == 0 else pexpO
                                oc = (h // 2) * 64
                                nc.tensor.matmul(
                                    den_t[0:64, h:h + 1],
                                    pex[0:KTOT, oc:oc + 64],
                                    ones_col_bf[0:KTOT, :],
                                    start=True, stop=True)
                            rec_t = asb.tile([64, NH], F32, tag="rec_t")
                            nc.vector.reciprocal_approx_fast(
                                out=rec_t[:], in_=den_t[:])
                            for h in range(NH):
                                hb = (h % 2) * 64
                                oc = (h // 2) * 64
                                pex = pexpE if h % 2 == 0 else pexpO
                                nc.tensor.matmul(
                                    av_t[hb:hb + 64, oc:oc + 64],
                                    Vn[0:KTOT, s * H + h * 64:
                                       s * H + (h + 1) * 64],
                                    pex[0:KTOT, oc:oc + 64],
                                    start=True, stop=True)
                            rec_tb = asb.tile([64, NH], BF16, tag="rec_tb")
                            nc.gpsimd.tensor_copy(rec_tb[:], rec_t[:])
                            rhp = dps.tile([NH, 64], BF16, tag="rhp")
                            nc.tensor.transpose(rhp[:], rec_tb[:],
                                                ident_bf[0:64, 0:64])
                            rec_h = asb.tile([NH, 64], BF16, tag="rec_h")
                            nc.vector.tensor_copy(rec_h[:], rhp[:])
                            st[s] = [av_t, rec_h]

                        def stage3(s):
                            av_t, rec_h = st.pop(s)
                            blk = s // SPB
                            spi = s % SPB
                            if spi == 0:
                                aob = asb.tile([128, NC_H * TBLK], FP8,
                                               tag="aob")
                                aobs[blk] = (aob,
                                             aob[:].rearrange(
                                                 "p (c n) -> p c n", c=NC_H))
                            aobv = aobs[blk][1]
                            # recT[p, c*64+q] = AS / den(head(c, p), q)
                            recT = rps.tile([128, 384], F32, tag="recT",
                                            bufs=1)
                            for c in range(NC_H):
                                nc.tensor.matmul(
                                    recT[:, c * 64:(c + 1) * 64],
                                    asel[:, c * 128:(c + 1) * 128],
                                    rec_h[:, :],
                                    start=True, stop=True)
                            recTs = asb.tile([128, 384], BF16, tag="recTs")
                            if s % 2 == 0:
                                nc.scalar.activation(recTs[:], recT[:],
                                                     AF.Copy)
                            else:
                                nc.vector.tensor_copy(recTs[:], recT[:])
                            nc.vector.tensor_tensor(
                                out=aobv[:, :, spi * SPAN:(spi + 1) * SPAN],
                                in0=av_t[:].rearrange(
                                    "p (c n) -> p c n", c=NC_H),
                                in1=recTs[:].rearrange(
                                    "p (c n) -> p c n", c=NC_H),
                                op=OP.mult)

                        def outproj(blk):
                            aobv = aobs.pop(blk)[1]
                            tb = blk * TBLK
                            if lidx == 1:
                                for oc in range(NC_H):
                                    ps = opps.tile([128, TBLK], F32,
                                                   tag="opps")
                                    if not bias0:
                                        nc.tensor.matmul(
                                            ps[:],
                                            bo1row[:, oc * 128:(oc + 1) * 128],
                                            ones_tb[:, :], start=True,
                                            stop=False)
                                    for j in range(NC_H // 2):
                                        nc.tensor.matmul(
                                            ps[:],
                                            wosv[:, 2 * j:2 * j + 2,
                                                 oc * 128:(oc + 1) * 128],
                                            aobv[:, 2 * j:2 * j + 2, :],
                                            start=(j == 0 and bias0),
                                            stop=(j == NC_H // 2 - 1),
                                            perf_mode=DR)
                                    # X1q = AS*X1 = ps/WS + Xbf  (fp8)
                                    nc.vector.scalar_tensor_tensor(
                                        out=X1q[:, oc * NTOK + tb:
                                                oc * NTOK + tb + TBLK],
                                        in0=ps[:], scalar=1.0 / WS,
                                        op0=OP.mult,
                                        in1=Xbf[:, oc * NTOK + tb:
                                                oc * NTOK + tb + TBLK],
                                        op1=OP.add)
                            else:
                                for tof in range(TBLK // 128):
                                    t = blk * (TBLK // 128) + tof
                                    for no in range(2):
                                        nw = 512 if no == 0 else 256
                                        ps = opps.tile([128, 512], F32,
                                                       tag="opps")
                                        for j in range(NC_H // 2):
                                            nc.tensor.matmul(
                                                ps[:, :nw],
                                                aobv[:, 2 * j:2 * j + 2,
                                                     tof * 128:(tof + 1) * 128],
                                                wosv[:, 2 * j:2 * j + 2,
                                                     no * 512:no * 512 + nw],
                                                start=(j == 0),
                                                stop=(j == NC_H // 2 - 1),
                                                perf_mode=DR)
                                        if bias0:
                                            nc.scalar.activation(
                                                ret[:, t * H + no * 512:
                                                    t * H + no * 512 + nw],
                                                ps[:, :nw], AF.Copy,
                                                scale=QCP * (TAG_RATE if fusedp4
                                                             else 1.0))
                                        else:
                                            nc.vector.scalar_tensor_tensor(
                                                out=ret[:, t * H + no * 512:
                                                        t * H + no * 512 + nw],
                                                in0=ps[:, :nw], scalar=QCP,
                                                op0=OP.mult,
                                                in1=bo2_bc[:,
                                                           no * 512:
                                                           no * 512 + nw],
                                                op1=OP.add)
                            if post_blk is not None:
                                post_blk(blk, ret)

                        if KPHASE >= g_at:
                            for it in range(NSP + 2):
                                if it < NSP:
                                    stage1(it)
                                if 1 <= it <= NSP:
                                    stage2(it - 1)
                                if 2 <= it <= NSP + 1:
                                    s3 = it - 2
                                    stage3(s3)
                                    if s3 % SPB == SPB - 1:
                                        outproj(s3 // SPB)
                    return ret

                # ============ Phase 2+3: the two attention layers ============
                X1q = big12("X1q")
                res2 = None
                fusedp4 = identg and an_trivial and KPHASE >= 10

                with tc.tile_pool(name="wqs", bufs=6) as wstr_s, \
                     tc.tile_pool(name="wvs", bufs=1) as wpool_s, \
                     tc.tile_pool(name="wos_s", bufs=1) as awstr_s:
                    wpools = (wstr_s, wpool_s, awstr_s)
                    with tc.tile_pool(name="att1", bufs=1) as lp1:
                        attention(1, Xt8, bq1, bk1, wqkv1q, woq1, bv1_bc, X1q,
                                  None, lp1, wpools)
                    xbf_cm.__exit__(None, None, None)
                    if KPHASE >= 7:
                        with tc.tile_pool(name="att2", bufs=1) as lp2, \
                             tc.tile_pool(name="tgkp", bufs=1) as tgkp:
                            tgKs = tgkp.tile([128, NC_H * NSP], BF16, tag="tgKs")
                            nc.sync.dma_start(
                                tgKs[:].rearrange("p (c n) -> p c n", c=NC_H),
                                tgKd[:].rearrange("(c p) n -> p c n", p=128))
                            res2 = attention(2, X1q, bq2, bk2, wqkv2q, woq2,
                                             bv2_bc, None, tgKs, lp2, wpools)
                pbigB_cm = tc.tile_pool(name="pbigB", bufs=1)
                pbigB = pbigB_cm.__enter__()
                out_emb = pbigB.tile([128, NT16 * H], BF16, tag="e24",
                                     bufs=1, name="out_emb")

                # ============ Phase 4 (fallback): scatter + LN1 ============
                with tc.tile_pool(name="p4sb", bufs=3) as p4, \
                     tc.tile_pool(name="p4ln", bufs=2) as p4ln, \
                     tc.tile_pool(name="p4ps", bufs=2, space="PSUM") as p4ps:
                    moff = 0
                    for c in range(0 if fusedp4 else
                                   (NT16 if KPHASE >= 10 else 0)):
                        raw = p4.tile([128, H], F32, tag="raw")
                        nc.sync.dma_start(raw[:], we[c * 128:(c + 1) * 128, :])
                        comb = p4.tile([128, H], F32, tag="comb")
                        ics = kranges[c]
                        if identg:
                            # spans tile the sequence: res2 rows ARE tokens
                            nc.vector.scalar_tensor_tensor(
                                out=comb[:, 0:512], in0=res2[:, c * H:c * H + 512],
                                scalar=TAG_RATE, in1=raw[:, 0:512],
                                op0=OP.mult, op1=OP.add)
                            nc.vector.scalar_tensor_tensor(
                                out=comb[:, 512:768],
                                in0=res2[:, c * H + 512:(c + 1) * H],
                                scalar=TAG_RATE, in1=raw[:, 512:768],
                                op0=OP.mult, op1=OP.add)
                        elif len(ics) > 0:
                            tps = p4ps.tile([128, 512], F32, tag="tps")
                            tps2 = p4ps.tile([128, 256], F32, tag="tps2")
                            for j, ic in enumerate(ics):
                                mb = p4.tile([128, 128], BF16, tag="mb")
                                nc.sync.dma_start(
                                    mb[:], mmatc[moff * 128:(moff + 1) * 128, :])
                                moff += 1
                                nc.tensor.matmul(tps[:], mb[:],
                                                 res2[:, ic * H: ic * H + 512],
                                                 start=(j == 0), stop=(j == len(ics) - 1))
                                nc.tensor.matmul(tps2[:], mb[:],
                                                 res2[:, ic * H + 512: (ic + 1) * H],
                                                 start=(j == 0), stop=(j == len(ics) - 1))
                            nc.vector.scalar_tensor_tensor(
                                out=comb[:, 0:512], in0=tps[:], scalar=TAG_RATE,
                                in1=raw[:, 0:512], op0=OP.mult, op1=OP.add)
                            nc.vector.scalar_tensor_tensor(
                                out=comb[:, 512:768], in0=tps2[:], scalar=TAG_RATE,
                                in1=raw[:, 512:768], op0=OP.mult, op1=OP.add)
                        else:
                            nc.gpsimd.tensor_copy(comb[:], raw[:])
                        _ln_apply(nc, p4ln, comb, slice(None), "ln1", ang_bc, anb_bc,
                                  an_trivial, eps_t, out_emb[:, c * H:(c + 1) * H])

                if f1:
                    oeT = big12("oeT8")
                    oescale = AS
                elif fusedp4:
                    # res2 is still live while oeT fills (interleaved with FFN)
                    oeT = pbigB.tile([128, NC_H * NTOK], BF16, tag="oet",
                                     bufs=1, name="oeT")
                    oescale = 1.0
                else:
                    oeT = big24("oeT")
                    oescale = 1.0
                oevw = oeT[:].rearrange("p (c n) -> p c n", c=NC_H)

                def oet_transpose(t, psp):
                    """transpose LN1-out tile t into oeT via PE (bf16)."""
                    tpT = psp.tile([128, H], BF16, tag="tpT", bufs=2)
                    for c in range(NC_H):
                        nc.tensor.transpose(
                            tpT[:, c * 128:(c + 1) * 128],
                            out_emb[:, t * H + c * 128: t * H + (c + 1) * 128],
                            ident_bf[:])
                    nc.scalar.activation(
                        oevw[:, 0:4, t * 128:(t + 1) * 128],
                        tpT[:, 0:512].rearrange("p (c n) -> p c n", c=4),
                        AF.Copy, scale=oescale)
                    if oescale == 1.0:
                        nc.vector.tensor_copy(
                            oevw[:, 4:6, t * 128:(t + 1) * 128],
                            tpT[:, 512:768].rearrange("p (c n) -> p c n", c=2))
                    else:
                        nc.vector.tensor_scalar_mul(
                            oevw[:, 4:6, t * 128:(t + 1) * 128],
                            tpT[:, 512:768].rearrange("p (c n) -> p c n", c=2),
                            oescale)

                # transpose out_emb -> oeT (non-fused fallback)
                with tc.tile_pool(name="p5ps2", bufs=2, space="PSUM") as p5ps:
                    for t in range(0 if fusedp4 else
                                   (NT16 if KPHASE >= 11 else 0)):
                        oet_transpose(t, p5ps)
                oev = oeT[:].rearrange("p (c n) -> p c n", c=NC_H)

                # ============ Phase 5: FFN + LN2 (in-place into out_emb) ========
                h1scale = (QCP if f1 else 1.0) * (AS if f2 else 1.0)
                ffscale = QCP if f2 else 1.0
                with tc.tile_pool(name="w5", bufs=1) as w5, \
                     tc.tile_pool(name="w5s", bufs=3) as w5s, \
                     tc.tile_pool(name="p45", bufs=1) as p45, \
                     tc.tile_pool(name="p45ln", bufs=2) as p45ln, \
                     tc.tile_pool(name="ffln", bufs=2) as ffln, \
                     tc.tile_pool(name="ffps", bufs=2, space="PSUM") as ffps, \
                     tc.tile_pool(name="ffps2", bufs=2, space="PSUM") as ffps2, \
                     tc.tile_pool(name="p6ps", bufs=1, space="PSUM") as p6ps, \
                     tc.tile_pool(name="p7ps", bufs=1, space="PSUM") as p7ps, \
                     tc.tile_pool(name="p7sb", bufs=2) as p7sb:
                    if KPHASE >= 14:
                        wout = w5.tile([128, NC_H * NL], BF16, tag="wout")
                        nc.sync.dma_start(
                            wout[:].rearrange("p (c n) -> p c n", c=NC_H),
                            woutT[:].rearrange("(c p) n -> p c n", p=128))
                    raw_tiles = {}

                    def prefetch_raw(cs):
                        for c in cs:
                            r = p45.tile([128, H], BF16, tag="raw",
                                         bufs=8, name=f"raw{c}")
                            nc.sync.dma_start(
                                r[:], web[c * 128:(c + 1) * 128, :])
                            raw_tiles[c] = r

                    def phase4_tile(c):
                        # merge scatter result + residual, LN1 in place;
                        # alternate DVE / Pool so the boundary drains fast
                        raw = raw_tiles.pop(c)
                        oe = out_emb[:, c * H:(c + 1) * H]
                        eng = nc.vector if c % 2 == 0 else nc.gpsimd
                        eng.tensor_tensor(
                            out=oe, in0=res2[:, c * H:(c + 1) * H],
                            in1=raw[:], op=OP.add)
                        _ln_fast(nc, p45ln, oe, "ln1", eps_t, oe,
                                 apply_eng=(nc.gpsimd if c % 2 == 0
                                            else nc.vector))

                    if fusedp4:
                        prefetch_raw(range(0, 8))
                    w2t = w5.tile([128, NC_I * H], W2DT, tag="w2t")
                    nc.sync.dma_start(w2t[:], w2d[:])
                    w2v = w2t[:].rearrange("p (c n) -> p c n", c=NC_I)
                    for tp in range(2 if KPHASE >= 12 else 0):
                        if fusedp4:
                            if tp == 0:
                                prefetch_raw(range(8, 16))
                            # scatter+LN1 for this half, then transpose into
                            # oeT — the second half rides under tp0's matmuls
                            for c in range(tp * 8, tp * 8 + 8):
                                phase4_tile(c)
                            for t in range(tp * 8, tp * 8 + 8):
                                oet_transpose(t, p6ps)
                        h1s = [pbigB.tile([128, NC_I * 512], H1DT, tag="h1",
                                          bufs=2, name=f"h1_{2 * tp + i}")
                               for i in range(2)]
                        for fc in range(NC_I):
                            w1s = w5s.tile([128, NC_H * 128], W1DT, tag="w1s")
                            nc.sync.dma_start(
                                w1s[:], w1d[:, fc * H:(fc + 1) * H])
                            w1v = w1s[:].rearrange("p (c n) -> p c n", c=NC_H)
                            for tki in range(2):
                                tkc = 2 * tp + tki
                                h1 = h1s[tki]
                                ps = ffps.tile([128, 512], F32, tag="h1ps")
                                if f1:
                                    for j in range(NC_H // 2):
                                        nc.tensor.matmul(
                                            ps[:], w1v[:, 2 * j:2 * j + 2, :],
                                            oev[:, 2 * j:2 * j + 2,
                                                tkc * 512:(tkc + 1) * 512],
                                            start=(j == 0),
                                            stop=(j == NC_H // 2 - 1),
                                            perf_mode=DR)
                                else:
                                    for ic in range(NC_H):
                                        nc.tensor.matmul(
                                            ps[:], w1s[:, ic * 128:(ic + 1) * 128],
                                            oeT[:, ic * NTOK + tkc * 512:
                                                ic * NTOK + (tkc + 1) * 512],
                                            start=(ic == 0), stop=(ic == NC_H - 1))
                                if bias0:
                                    if fc % 2 == 0:
                                        nc.vector.tensor_scalar(
                                            out=h1[:, fc * 512:(fc + 1) * 512],
                                            in0=ps[:], scalar1=h1scale, scalar2=0.0,
                                            op0=OP.mult, op1=OP.max)
                                    else:
                                        nc.scalar.activation(
                                            h1[:, fc * 512:(fc + 1) * 512],
                                            ps[:], AF.Relu, scale=h1scale)
                                else:
                                    nc.scalar.activation(
                                        h1[:, fc * 512:(fc + 1) * 512],
                                        ps[:], AF.Relu, bias=b1t[:, fc:fc + 1],
                                        scale=h1scale)
                        for tt8 in range(8):
                            tkc = 2 * tp + tt8 // 4
                            tt = tt8 % 4
                            h1 = h1s[tt8 // 4]
                            h1v = h1[:].rearrange("p (c n) -> p c n", c=NC_I)
                            t = tkc * 4 + tt
                            for no in range(2):
                                nw = 512 if no == 0 else 256
                                ps2 = ffps2.tile([128, 512], F32, tag="ffout")
                                if not bias0:
                                    # b2 via ones x b2row init
                                    nc.tensor.matmul(
                                        ps2[:, :nw], ones_row_bf[:, :],
                                        b2row[:, no * 512:no * 512 + nw],
                                        start=True, stop=False)
                                if f2:
                                    for j in range(NC_I // 2):
                                        nc.tensor.matmul(
                                            ps2[:, :nw],
                                            h1v[:, 2 * j:2 * j + 2,
                                                tt * 128:(tt + 1) * 128],
                                            w2v[:, 2 * j:2 * j + 2,
                                                no * 512:no * 512 + nw],
                                            start=(j == 0 and bias0),
                                            stop=(j == NC_I // 2 - 1),
                                            perf_mode=DR)
                                else:
                                    for fc in range(NC_I):
                                        nc.tensor.matmul(
                                            ps2[:, :nw],
                                            h1[:, fc * 512 + tt * 128:
                                               fc * 512 + (tt + 1) * 128],
                                            w2t[:, fc * H + no * 512:
                                                fc * H + no * 512 + nw],
                                            start=(fc == 0 and bias0),
                                            stop=(fc == NC_I - 1))
                                nc.vector.scalar_tensor_tensor(
                                    out=out_emb[:, t * H + no * 512:
                                                t * H + no * 512 + nw],
                                    in0=ps2[:, :nw], scalar=ffscale, op0=OP.mult,
                                    in1=out_emb[:, t * H + no * 512:
                                                t * H + no * 512 + nw],
                                    op1=OP.add)
                            if fn_trivial:
                                _ln_fast(nc, ffln, out_emb[:, t * H:(t + 1) * H],
                                         "ln2", eps_t,
                                         out_emb[:, t * H:(t + 1) * H])
                            else:
                                _ln_apply(nc, ffln, out_emb,
                                          slice(t * H, (t + 1) * H), "ln2",
                                          fng_bc, fnb_bc, fn_trivial, eps_t,
                                          out_emb[:, t * H:(t + 1) * H])
                            if KPHASE < 13:
                                continue
                            # fused tail: transpose ln2(t) -> ln2T, Wout matmul,
                            # store — overlaps the old post-FFN tail into the
                            # PE-busy FFN window
                            tpT = p6ps.tile([128, H], BF16, tag="tpT", bufs=2)
                            for c in range(NC_H):
                                nc.tensor.transpose(
                                    tpT[:, c * 128:(c + 1) * 128],
                                    out_emb[:, t * H + c * 128: t * H + (c + 1) * 128],
                                    ident_bf[:])
                            lt = p7sb.tile([128, H], BF16, tag="l2t", bufs=3)
                            nc.scalar.activation(lt[:, 0:512], tpT[:, 0:512],
                                                 AF.Copy)
                            nc.vector.tensor_copy(lt[:, 512:768], tpT[:, 512:768])
                            if KPHASE >= 14:
                                ps3 = p7ps.tile([128, NL], F32, tag="outps")
                                for ic in range(NC_H):
                                    nc.tensor.matmul(
                                        ps3[:],
                                        lt[:, ic * 128:(ic + 1) * 128],
                                        wout[:, ic * NL:(ic + 1) * NL],
                                        start=(ic == 0), stop=(ic == NC_H - 1))
                                ot = p7sb.tile([128, NL], F32, tag="ot")
                                if bias0:
                                    nc.vector.tensor_copy(ot[:], ps3[:])
                                else:
                                    nc.vector.tensor_tensor(
                                        out=ot[:], in0=ps3[:],
                                        in1=bout_bc[:, :NL], op=OP.add)
                                nc.sync.dma_start(out_d[t * 128:(t + 1) * 128, :],
                                                  ot[:])
                    if KPHASE < 14:
                        for t in range(NT16):
                            nc.sync.dma_start(out_d[t * 128:(t + 1) * 128, :],
                                              we[t * 128:(t + 1) * 128, 0:NL])
                pbigB_cm.__exit__(None, None, None)

    return nc


def _host_prep(word_embedding, span_starts, span_tags, tag_emb,
               sa_Wqkv, sa_bqkv, sa_Wo, sa_bo,
               at_Wqkv, at_bqkv, at_Wo, at_bo,
               an_g, an_b, W1, b1, W2, b2, fn_g, fn_b, Wout, bout):
    bf = ml_dtypes.bfloat16
    f8 = ml_dtypes.float8_e4m3
    we = np.ascontiguousarray(np.asarray(word_embedding, dtype=np.float32))
    starts = np.asarray(span_starts).astype(np.int64)
    tags = np.asarray(span_tags).astype(np.int64)
    pos = (starts[:, :, None] + np.arange(SPAN)[None, None, :]).reshape(B, NTOK)

    half = H // 2
    inv = 1.0 / (10000.0 ** (np.arange(half, dtype=np.float64) / half))
    ang = np.arange(S, dtype=np.float64)[:, None] * inv[None, :]
    cos_t = np.cos(ang).astype(np.float32)
    sin_t = np.sin(ang).astype(np.float32)

    # scatter selection blocks + union k-ranges
    kr_lo = np.full(NT16, NTOK, dtype=np.int64)
    kr_hi = np.zeros(NT16, dtype=np.int64)
    for b in range(B):
        for c in range(NT16):
            js = np.nonzero((starts[b] < (c + 1) * 128) & (starts[b] + SPAN > c * 128))[0]
            if len(js) > 0:
                kr_lo[c] = min(kr_lo[c], js[0] * SPAN)
                kr_hi[c] = max(kr_hi[c], (js[-1] + 1) * SPAN)
    kranges = []
    for c in range(NT16):
        if kr_hi[c] <= kr_lo[c]:
            kranges.append(())
        else:
            kranges.append(tuple(range(int(kr_lo[c]) // 128,
                                       int(np.ceil(kr_hi[c] / 128)))))
    kranges = tuple(kranges)

    f32 = lambda x: np.ascontiguousarray(np.asarray(x, dtype=np.float32))
    tobf = lambda x: np.ascontiguousarray(np.asarray(x, dtype=np.float32).astype(bf))
    tof8 = lambda x: np.ascontiguousarray(np.asarray(x, dtype=np.float32).astype(f8))
    sa_Wqkv, sa_bqkv, sa_Wo, sa_bo = f32(sa_Wqkv), f32(sa_bqkv), f32(sa_Wo), f32(sa_bo)
    at_Wqkv, at_bqkv, at_Wo, at_bo = f32(at_Wqkv), f32(at_bqkv), f32(at_Wo), f32(at_bo)
    an_g, an_b, fn_g, fn_b = f32(an_g), f32(an_b), f32(fn_g), f32(fn_b)
    W1, b1, W2, b2, Wout, bout = f32(W1), f32(b1), f32(W2), f32(b2), f32(Wout), f32(bout)
    tag_emb = f32(tag_emb)
    f1 = KFFN in ("fp8", "mix1")
    f2 = KFFN in ("fp8", "mix2")

    def qkv_q(Wqkv):
        # [3H, H] -> [128, (qk, oc, c, n) | (c, n768)] fp8: Q section
        # pre-scaled by SCALE*WS, K/V by WS, pre-layouted so each kernel
        # stream DMA is contiguous on both sides
        wT = Wqkv.T.copy()
        wT[:, 0:H] *= SCALE * WS
        wT[:, H:] *= WS
        qk = (wT[:, :2 * H].reshape(NC_H, 128, 2, NC_H, 128)
              .transpose(1, 2, 3, 0, 4).reshape(128, -1))
        v = (wT[:, 2 * H:].reshape(NC_H, 128, H)
             .transpose(1, 0, 2).reshape(128, -1))
        return tof8(np.concatenate([qk, v], axis=1))

    def wo_q(Wo):
        return tof8((Wo.T * WS).reshape(NC_H, 128, H)
                    .transpose(1, 0, 2).reshape(128, -1))

    shared = {
        "wq1p": qkv_q(sa_Wqkv), "wo1p": wo_q(sa_Wo),
        "wq2p": qkv_q(at_Wqkv), "wo2p": wo_q(at_Wo),
        "woutT": tobf(Wout.T),
        "bq1c": np.ascontiguousarray((sa_bqkv[0:H] * SCALE).reshape(NC_H, 128).T),
        "bk1c": np.ascontiguousarray(sa_bqkv[H:2 * H].reshape(NC_H, 128).T),
        "bq2c": np.ascontiguousarray((at_bqkv[0:H] * SCALE).reshape(NC_H, 128).T),
        "bk2c": np.ascontiguousarray(at_bqkv[H:2 * H].reshape(NC_H, 128).T),
        "vec_bo1r": (sa_bo * AS * WS).reshape(1, H),
        "b1c": np.ascontiguousarray(
            (b1 * (AS if f2 else 1.0)).reshape(NC_I, 128).T),
        "vec_bv1": sa_bqkv[2 * H:].reshape(1, H),
        "vec_bv2": at_bqkv[2 * H:].reshape(1, H),
        "vec_bo2": at_bo.reshape(1, H),
        "vec_b2": (b2 * (AS * WS if f2 else 1.0)).reshape(1, H),
        "vec_ang": an_g.reshape(1, H), "vec_anb": an_b.reshape(1, H),
        "vec_fng": fn_g.reshape(1, H), "vec_fnb": fn_b.reshape(1, H),
        "vec_bout": bout.reshape(1, NL),
    }
    w1p = (W1.T * (WS if f1 else 1.0)).reshape(NC_H, 128, NC_I, 128) \
        .transpose(1, 2, 0, 3).reshape(128, -1)
    w2p = (W2.T * (WS if f2 else 1.0)).reshape(NC_I, 128, H) \
        .transpose(1, 0, 2).reshape(128, -1)
    shared["w1pp"] = tof8(w1p) if f1 else tobf(w1p)
    shared["w2pp"] = tof8(w2p) if f2 else tobf(w2p)
    an_trivial = bool(np.all(an_g == 1.0) and np.all(an_b == 0.0))
    fn_trivial = bool(np.all(fn_g == 1.0) and np.all(fn_b == 0.0))
    bias0 = bool(all(np.all(v == 0.0) for v in
                     (sa_bqkv, sa_bo, at_bqkv, at_bo, b1, b2, bout)))
    identg = bool(np.all(pos == np.arange(NTOK)[None, :]))

    at_Wk = at_Wqkv[H:2 * H]
    at_Wv = at_Wqkv[2 * H:]
    bk2 = at_bqkv[H:2 * H]
    bv2 = at_bqkv[2 * H:]

    in_maps = []
    for b in range(B):
        pb = pos[b]
        tagv = tag_emb[tags[b]]  # [NSP, H]
        tagK = tagv @ at_Wk.T + bk2
        tagV = tagv @ at_Wv.T + bv2
        blocks = []
        for c in range(NT16):
            for ic in kranges[c]:
                m = (pb[ic * 128:(ic + 1) * 128, None]
                     == np.arange(c * 128, (c + 1) * 128)[None, :])
                blocks.append(m.astype(bf))
        mmatc = (np.concatenate(blocks, axis=0) if blocks
                 else np.zeros((128, 128), bf))
        m = {
            "we": we[b],
            "web": np.ascontiguousarray(we[b].astype(bf)),
            "posi": np.ascontiguousarray(pb.reshape(NT16, 128).T.astype(np.int32)),
            "cosg": np.ascontiguousarray((AS * cos_t[pb]).astype(bf)),
            "sing": np.ascontiguousarray((AS * sin_t[pb]).astype(bf)),
            "mmatc": np.ascontiguousarray(mmatc),
            "tgKd": np.ascontiguousarray(tagK.T.astype(bf)),
            "tgVd": np.ascontiguousarray(tagV.reshape(1, NSP * H).astype(bf)),
        }
        m.update(shared)
        in_maps.append(m)
    return in_maps, kranges, an_trivial, fn_trivial, bias0, identg


def kernel(**inputs):
    in_maps, kranges, an_trivial, fn_trivial, bias0, identg = _host_prep(**inputs)
    key = (kranges, an_trivial, fn_trivial, bias0, identg, KFFN)
    if key not in _CACHE:
        _CACHE[key] = build(kranges, an_trivial, fn_trivial, bias0, identg)
    nc = _CACHE[key]
    res = run_bass_kernel_spmd(nc, in_maps, core_ids=list(range(8)), trace=False)
    out = np.stack([res.results[b]["out"] for b in range(B)], axis=0)
    return out.astype(np.float32)


if __name__ == "__main__":
    import reference
    inputs = {k: np.asarray(v) for k, v in reference.setup_inputs().items()}
    exp = np.asarray(reference.reference(**inputs))
    got = kernel(**inputs)
    err = np.abs(got - exp).max()
    rel = err / np.abs(exp).max()
    print("max abs err:", err, "rel:", rel)



# revision 40
# speedup vs baseline: 1.0291x; 1.0291x over previous
"""Trainium2 Bass kernel for nn_Estor_45595372814585 (span transformer block).

Sharding: batch b -> NeuronCore b (8 batches, 8 cores), no collectives.
QKV / out-proj matmuls in fp8e4 with DoubleRow perf mode (2 k-chunks per
instruction); scores/softmax/AV in bf16; FFN selectable bf16 or fp8
(KFFN env). Residuals/LayerNorm in fp32.

Scaling convention: activations entering fp8 matmuls are pre-scaled by
AS, weights by WS (Q weights additionally by SCALE). PSUM therefore
holds AS*WS*(x@w); copy-outs divide back.
"""
import sys, os
sys.path.insert(0, '/opt/trn_rl_repo')
KFFN = os.environ.get("KFFN", "mix2")     # "bf16" | "fp8" | "mix1" | "mix2"
KPHASE = int(os.environ.get("KPHASE", "99"))
import numpy as np
import ml_dtypes

from concourse import bass, bacc, mybir, tile
from concourse.bass_utils import run_bass_kernel_spmd
from concourse.masks import make_identity

F32 = mybir.dt.float32
BF16 = mybir.dt.bfloat16
FP8 = mybir.dt.float8e4
I32 = mybir.dt.int32
AF = mybir.ActivationFunctionType
OP = mybir.AluOpType
DR = mybir.MatmulPerfMode.DoubleRow

B, S, H, NH, NT, NL, I = 8, 2048, 768, 12, 32, 9, 3072
SPAN, NSP, DH = 64, 32, 64
TAG_RATE = 2.0
EPS = 1e-12
NC_H = H // 128          # 6 hidden chunks
NC_I = I // 128          # 24 ffn chunks
NTOK = NSP * SPAN        # 2048 gathered tokens
NT16 = NTOK // 128       # 16 token tiles
SCALE = 1.0 / (DH ** 0.5)
AS = 8.0                 # activation fp8 pre-scale
WS = 32.0                # weight fp8 pre-scale
SKEY2 = SPAN + 1         # layer-2 keys per span (64 tokens + tag)

NBLK = 4                 # span blocks for the fused out-projections
SPB = NSP // NBLK        # 8 spans / block
TBLK = SPB * SPAN        # 512 tokens / block

_CACHE = {}


def _bcast_vec(nc, ps_pool, sb_pool, ones_f32, src_dram, n, tag, row_pool=None):
    """Broadcast a [1, n] f32 DRAM vector to a [128, n] f32 SBUF tile via PE."""
    row = (row_pool or sb_pool).tile([1, n], F32, tag="rowtmp", bufs=2, name="rowtmp")
    nc.sync.dma_start(row[:], src_dram[:])
    out = sb_pool.tile([128, n], F32, tag=tag + "_bc", bufs=1)
    for o in range(0, n, 512):
        w = min(512, n - o)
        p = ps_pool.tile([128, 512], F32, tag="bcps")
        nc.tensor.matmul(p[:, :w], ones_f32[:, :], row[:, o:o + w], start=True, stop=True)
        nc.vector.tensor_copy(out[:, o:o + w], p[:, :w])
    return out


def _ln_apply(nc, sb, x_tile, cols, scratch_tag, g_bc, b_bc, trivial, eps_t, out_ap):
    """LayerNorm over free-dim H on x_tile[:, cols] (f32, [128, 768]);
    writes out_ap (f32)."""
    mean = sb.tile([128, 1], F32, tag=scratch_tag + "_m")
    negm = sb.tile([128, 1], F32, tag=scratch_tag + "_nm")
    ss = sb.tile([128, 1], F32, tag=scratch_tag + "_ss")
    sq = sb.tile([128, H], F32, tag=scratch_tag + "_sq")
    rstd = sb.tile([128, 1], F32, tag=scratch_tag + "_rs")
    xin = x_tile[:, cols]
    nc.vector.reduce_sum(out=mean[:], in_=xin, axis=mybir.AxisListType.X)
    nc.vector.tensor_scalar_mul(negm[:], mean[:], -1.0 / H)
    sd = sb.tile([128, 1], F32, tag=scratch_tag + "_sd")
    nc.scalar.activation(sq[:], xin, AF.Square, bias=negm[:, :1], accum_out=ss[:])
    nc.scalar.activation(sd[:], ss[:], AF.Sqrt, bias=eps_t[:, :1], scale=1.0 / H)
    nc.vector.reciprocal_approx_fast(out=rstd[:], in_=sd[:])
    if trivial:
        nc.gpsimd.tensor_scalar(out=out_ap, in0=xin, scalar1=negm[:, :1],
                                scalar2=rstd[:, :1], op0=OP.add, op1=OP.mult)
    else:
        nc.gpsimd.tensor_scalar(out=sq[:], in0=xin, scalar1=negm[:, :1],
                                scalar2=rstd[:, :1], op0=OP.add, op1=OP.mult)
        nc.vector.tensor_tensor(out=sq[:], in0=sq[:], in1=g_bc[:], op=OP.mult)
        nc.vector.tensor_tensor(out=out_ap, in0=sq[:], in1=b_bc[:], op=OP.add)


def _ln_fast(nc, sb, xin, tag, eps_t, out_ap, apply_eng=None):
    """Trivial LayerNorm (g=1, b=0) with independent mean / E[x^2]
    reductions so neither engine waits on the other: var = E[x^2]-m^2."""
    ssum = sb.tile([128, 1], F32, tag=tag + "_s")
    negm = sb.tile([128, 1], F32, tag=tag + "_nm")
    msq = sb.tile([128, 1], F32, tag=tag + "_m2")
    ss = sb.tile([128, 1], F32, tag=tag + "_ss")
    var = sb.tile([128, 1], F32, tag=tag + "_v")
    sd = sb.tile([128, 1], F32, tag=tag + "_sd")
    rstd = sb.tile([128, 1], F32, tag=tag + "_rs")
    sq = sb.tile([128, H], BF16, tag=tag + "_sq")
    nc.vector.reduce_sum(out=ssum[:], in_=xin, axis=mybir.AxisListType.X)
    nc.scalar.activation(sq[:], xin, AF.Square, accum_out=ss[:])
    nc.vector.tensor_scalar_mul(negm[:], ssum[:], -1.0 / H)
    nc.vector.tensor_tensor(out=msq[:], in0=negm[:], in1=negm[:], op=OP.mult)
    nc.vector.scalar_tensor_tensor(out=var[:], in0=ss[:], scalar=1.0 / H,
                                   op0=OP.mult, in1=msq[:], op1=OP.subtract)
    nc.scalar.activation(sd[:], var[:], AF.Sqrt, bias=eps_t[:, :1])
    nc.vector.reciprocal_approx_fast(out=rstd[:], in_=sd[:])
    (apply_eng or nc.gpsimd).tensor_scalar(
        out=out_ap, in0=xin, scalar1=negm[:, :1],
        scalar2=rstd[:, :1], op0=OP.add, op1=OP.mult)


def build(kranges, an_trivial, fn_trivial, bias0=True, identg=False, ffn_mode=KFFN):
    nc = _build_ir(kranges, an_trivial, fn_trivial, bias0, identg, ffn_mode)
    nc.compile()
    return nc


def _build_ir(kranges, an_trivial, fn_trivial, bias0, identg, ffn_mode):
    nc = bacc.Bacc("TRN2", target_bir_lowering=False, debug=False, num_devices=8)
    f1 = ffn_mode in ("fp8", "mix1")   # stage-1 (W1) matmul in fp8
    f2 = ffn_mode in ("fp8", "mix2")   # stage-2 (W2) matmul in fp8
    NPAIR = sum(len(r) for r in kranges)

    # ---- DRAM I/O ----
    we = nc.dram_tensor("we", [S, H], F32, kind="ExternalInput")
    web = nc.dram_tensor("web", [S, H], BF16, kind="ExternalInput")
    posi = nc.dram_tensor("posi", [128, NT16], I32, kind="ExternalInput")
    cosg = nc.dram_tensor("cosg", [NTOK, H // 2], BF16, kind="ExternalInput")
    sing = nc.dram_tensor("sing", [NTOK, H // 2], BF16, kind="ExternalInput")
    mmatc = nc.dram_tensor("mmatc", [max(NPAIR, 1) * 128, 128], BF16,
                           kind="ExternalInput")
    tgKd = nc.dram_tensor("tgKd", [H, NSP], BF16, kind="ExternalInput")
    tgVd = nc.dram_tensor("tgVd", [1, NSP * H], BF16, kind="ExternalInput")
    # weights pre-layouted on host to [128, ...] so every stream DMA is a
    # contiguous >=512B-per-descriptor copy
    wqkv1q = nc.dram_tensor("wq1p", [128, 18 * H], FP8, kind="ExternalInput")
    wqkv2q = nc.dram_tensor("wq2p", [128, 18 * H], FP8, kind="ExternalInput")
    woq1 = nc.dram_tensor("wo1p", [128, NC_H * H], FP8, kind="ExternalInput")
    woq2 = nc.dram_tensor("wo2p", [128, NC_H * H], FP8, kind="ExternalInput")
    W1DT = FP8 if f1 else BF16
    W2DT = FP8 if f2 else BF16
    H1DT = FP8 if f2 else BF16
    w1d = nc.dram_tensor("w1pp", [128, NC_I * NC_H * 128], W1DT,
                         kind="ExternalInput")
    w2d = nc.dram_tensor("w2pp", [128, NC_I * H], W2DT,
                         kind="ExternalInput")
    woutT = nc.dram_tensor("woutT", [H, NL], BF16, kind="ExternalInput")
    # bias columns ([128, n_chunks] f32, chunk c in col c)
    bq1c = nc.dram_tensor("bq1c", [128, NC_H], F32, kind="ExternalInput")  # SCALE*bq1
    bk1c = nc.dram_tensor("bk1c", [128, NC_H], F32, kind="ExternalInput")  # bk1
    bq2c = nc.dram_tensor("bq2c", [128, NC_H], F32, kind="ExternalInput")  # SCALE*bq2
    bk2c = nc.dram_tensor("bk2c", [128, NC_H], F32, kind="ExternalInput")  # bk2
    vec_bo1r = nc.dram_tensor("vec_bo1r", [1, H], F32, kind="ExternalInput")  # AS*WS*bo1
    b1c = nc.dram_tensor("b1c", [128, NC_I], F32, kind="ExternalInput")    # (AS*)b1
    # bias rows
    vec_bv1 = nc.dram_tensor("vec_bv1", [1, H], F32, kind="ExternalInput")
    vec_bv2 = nc.dram_tensor("vec_bv2", [1, H], F32, kind="ExternalInput")
    vec_bo2 = nc.dram_tensor("vec_bo2", [1, H], F32, kind="ExternalInput")
    vec_b2 = nc.dram_tensor("vec_b2", [1, H], F32, kind="ExternalInput")   # (AS*WS*)b2
    vec_ang = nc.dram_tensor("vec_ang", [1, H], F32, kind="ExternalInput")
    vec_anb = nc.dram_tensor("vec_anb", [1, H], F32, kind="ExternalInput")
    vec_fng = nc.dram_tensor("vec_fng", [1, H], F32, kind="ExternalInput")
    vec_fnb = nc.dram_tensor("vec_fnb", [1, H], F32, kind="ExternalInput")
    vec_bout = nc.dram_tensor("vec_bout", [1, NL], F32, kind="ExternalInput")
    out_d = nc.dram_tensor("out", [S, NL], F32, kind="ExternalOutput")

    QCP = 1.0 / (AS * WS)   # generic fp8 psum descale

    with tile.TileContext(nc) as tc:
        with tc.tile_pool(name="const", bufs=1) as csb:
            ones_f32 = csb.tile([1, 128], F32, tag="ones_f32")
            nc.vector.memset(ones_f32[:], 1.0)
            ones_col_bf = csb.tile([128, 1], BF16, tag="ones_col_bf")
            nc.vector.memset(ones_col_bf[:], 1.0)
            ones_row_bf = csb.tile([1, 128], BF16, tag="ones_row_bf")
            nc.vector.memset(ones_row_bf[:], 1.0)
            as12 = csb.tile([NH, 64], BF16, tag="as12")
            nc.vector.memset(as12[:], AS)
            ones_tb = None
            if not bias0:
                ones_tb = csb.tile([1, TBLK], BF16, tag="ones_tb")
                nc.vector.memset(ones_tb[:], 1.0)
            ident = csb.tile([128, 128], F32, tag="ident")
            make_identity(nc, ident[:])
            ident_bf = csb.tile([128, 128], BF16, tag="ident_bf")
            nc.vector.tensor_copy(ident_bf[:], ident[:])
            # asel[h, j*64+f] = AS if h == j else 0 — selector for broadcasting
            # one head-row of rec_h across 64 feature partitions
            asel = csb.tile([NH, NH * 64], BF16, tag="asel")
            for j in range(NH):
                nc.vector.tensor_scalar(
                    out=asel[:, j * 64:(j + 1) * 64], in0=as12[:, :],
                    scalar1=ident[0:NH, j:j + 1], scalar2=None, op0=OP.mult)
            eps_t = csb.tile([128, 1], F32, tag="eps")
            nc.vector.memset(eps_t[:], EPS)
            bq1 = bk1 = bq2 = bk2 = b1t = bo1row = None
            if not bias0:
                bq1 = csb.tile([128, NC_H], F32, tag="bq1")
                nc.sync.dma_start(bq1[:], bq1c[:])
                bk1 = csb.tile([128, NC_H], F32, tag="bk1")
                nc.sync.dma_start(bk1[:], bk1c[:])
                bq2 = csb.tile([128, NC_H], F32, tag="bq2")
                nc.sync.dma_start(bq2[:], bq2c[:])
                bk2 = csb.tile([128, NC_H], F32, tag="bk2")
                nc.sync.dma_start(bk2[:], bk2c[:])
                b1t = csb.tile([128, NC_I], F32, tag="b1t")
                nc.sync.dma_start(b1t[:], b1c[:])
                bo1row_f = csb.tile([1, H], F32, tag="bo1row_f")
                nc.sync.dma_start(bo1row_f[:], vec_bo1r[:])
                bo1row = csb.tile([1, H], BF16, tag="bo1row")
                nc.vector.tensor_copy(bo1row[:], bo1row_f[:])

            with tc.tile_pool(name="cps", bufs=1, space="PSUM") as cps, \
                 tc.tile_pool(name="crow", bufs=1) as crow:
                bv1_bc = bv2_bc = bo2_bc = bout_bc = None
                if not bias0:
                    bv1_bc = _bcast_vec(nc, cps, csb, ones_f32, vec_bv1, H, "bv1",
                                        row_pool=crow)
                    bv2_bc = _bcast_vec(nc, cps, csb, ones_f32, vec_bv2, H, "bv2",
                                        row_pool=crow)
                    bo2_bc = _bcast_vec(nc, cps, csb, ones_f32, vec_bo2, H, "bo2",
                                        row_pool=crow)
                    bout_bc = _bcast_vec(nc, cps, csb, ones_f32, vec_bout, NL, "bout",
                                         row_pool=crow)
                ang_bc = anb_bc = fng_bc = fnb_bc = None
                if not an_trivial:
                    ang_bc = _bcast_vec(nc, cps, csb, ones_f32, vec_ang, H, "ang", row_pool=crow)
                    anb_bc = _bcast_vec(nc, cps, csb, ones_f32, vec_anb, H, "anb", row_pool=crow)
                if not fn_trivial:
                    fng_bc = _bcast_vec(nc, cps, csb, ones_f32, vec_fng, H, "fng", row_pool=crow)
                    fnb_bc = _bcast_vec(nc, cps, csb, ones_f32, vec_fnb, H, "fnb", row_pool=crow)
                b2row = None
                if not bias0:
                    b2row_f = crow.tile([1, H], F32, tag="b2row_f")
                    nc.sync.dma_start(b2row_f[:], vec_b2[:])
                    b2row = csb.tile([1, H], BF16, tag="b2row")
                    nc.vector.tensor_copy(b2row[:], b2row_f[:])

            # ---------------- shared big SBUF slots ----------------
            with tc.tile_pool(name="pbig", bufs=1) as pbig:
                def big24(name):
                    return pbig.tile([128, NC_H * NTOK], BF16, tag="p24", bufs=1,
                                     name=name)

                def big12(name):
                    return pbig.tile([128, NC_H * NTOK], FP8, tag="p12", bufs=2,
                                     name=name)

                # ========= Phase 1: gather + rope(->AS-scaled) + transpose =========
                Xt8 = big12("Xt8")            # AS * rope(x), fp8, T-layout
                xbf_cm = tc.tile_pool(name="xbfp", bufs=1, side="right")
                xbfp = xbf_cm.__enter__()
                # AS * (rope(x) + bo1), bf16, T-layout; dies after attention 1
                Xbf = xbfp.tile([128, NC_H * NTOK], BF16, tag="xbf", bufs=1,
                                name="Xbf")
                with tc.tile_pool(name="p1sb", bufs=4) as p1, \
                     tc.tile_pool(name="p1ps", bufs=2, space="PSUM") as p1ps:
                    if not identg:
                        idx_t = p1.tile([128, NT16], I32, tag="idx", bufs=1)
                        nc.sync.dma_start(idx_t[:], posi[:])
                    for t in range(NT16):
                        g = p1.tile([128, H], BF16, tag="g")
                        if identg:
                            nc.sync.dma_start(g[:], web[t * 128:(t + 1) * 128, :])
                        else:
                            nc.gpsimd.indirect_dma_start(
                                out=g[:], out_offset=None, in_=web[:],
                                in_offset=bass.IndirectOffsetOnAxis(
                                    ap=idx_t[:, t:t + 1], axis=0))
                        cos_t = p1.tile([128, H // 2], BF16, tag="cos")
                        sin_t = p1.tile([128, H // 2], BF16, tag="sin")
                        nc.sync.dma_start(cos_t[:], cosg[t * 128:(t + 1) * 128, :])
                        nc.sync.dma_start(sin_t[:], sing[t * 128:(t + 1) * 128, :])
                        # cos/sin tables carry the AS prescale (host-side)
                        ge = g[:, 0:H:2]
                        go = g[:, 1:H:2]
                        t0 = p1.tile([128, H // 2], BF16, tag="t0")
                        t1 = p1.tile([128, H // 2], BF16, tag="t1")
                        rp = p1.tile([128, H], BF16, tag="rp")
                        nc.vector.tensor_tensor(out=t0[:], in0=ge, in1=cos_t[:], op=OP.mult)
                        nc.gpsimd.tensor_tensor(out=t1[:], in0=go, in1=sin_t[:], op=OP.mult)
                        nc.vector.tensor_tensor(out=rp[:, 0:H:2], in0=t0[:], in1=t1[:],
                                                op=OP.subtract)
                        nc.gpsimd.tensor_tensor(out=t0[:], in0=ge, in1=sin_t[:], op=OP.mult)
                        nc.vector.tensor_tensor(out=t1[:], in0=go, in1=cos_t[:], op=OP.mult)
                        nc.gpsimd.tensor_tensor(out=rp[:, 1:H:2], in0=t0[:], in1=t1[:],
                                                op=OP.add)
                        trp = p1ps.tile([128, H], BF16, tag="trp")
                        for c in range(NC_H):
                            nc.tensor.transpose(trp[:, c * 128:(c + 1) * 128],
                                                rp[:, c * 128:(c + 1) * 128],
                                                ident_bf[:])
                        trpv = trp[:].rearrange("p (c n) -> p c n", c=NC_H)
                        x8v = Xt8[:].rearrange("p (c n) -> p c n", c=NC_H)
                        xbv = Xbf[:].rearrange("p (c n) -> p c n", c=NC_H)
                        nc.scalar.activation(
                            x8v[:, :, t * 128:(t + 1) * 128], trpv[:, :, :],
                            AF.Copy)
                        nc.vector.tensor_copy(
                            xbv[:, :, t * 128:(t + 1) * 128], trpv[:, :, :])

                # ============ attention layer ============
                def attention(lidx, Xin8, bq, bk, wqkvT, woT, bv_bc, X1q, tgKs, lp,
                              wpools, post_blk=None):
                    """lidx 1: self-attn over spans; returns nothing (writes X1q).
                    lidx 2: tag-augmented attn; returns res2 (natural layout).
                    lp: attention-local tile pool (Q/K/V die at return)."""
                    if lidx == 1:
                        g_qk, g_v, g_at, g_av, g_op = 2, 3, 4, 5, 6
                    else:
                        g_qk, g_v, g_at, g_av, g_op = 7, 7, 8, 8, 9
                    KTOT = SPAN if lidx == 1 else SKEY2
                    Qt = lp.tile([128, NC_H * NTOK], BF16, tag="qt", bufs=1)
                    if lidx == 1:
                        Kt = lp.tile([128, NC_H * NTOK], BF16, tag="kt", bufs=1)
                        ktv = None
                    else:
                        Kt = lp.tile([128, NC_H * NSP * SKEY2], BF16, tag="kt2",
                                     bufs=1)
                        ktv = Kt[:].rearrange("p (c s k) -> p c s k", c=NC_H, k=SKEY2)
                    Vn = lp.tile([SKEY2, NSP * H], BF16, tag="vn", bufs=1)
                    Xv = Xin8[:].rearrange("p (c n) -> p c n", c=NC_H)
                    wstr, wpool, awstr = wpools
                    with tc.tile_pool(name=f"qkvp{lidx}", bufs=3, space="PSUM") as qps, \
                         tc.tile_pool(name=f"qkvv{lidx}", bufs=2, space="PSUM") as vqps:
                        for qk, bcol in (((0, bq), (1, bk)) if KPHASE >= g_qk else ()):
                            for oc in range(NC_H):
                                wqo = wstr.tile([128, NC_H * 128], FP8, tag="wqo")
                                nc.sync.dma_start(
                                    wqo[:],
                                    wqkvT[:, (qk * NC_H + oc) * H:
                                          (qk * NC_H + oc + 1) * H])
                                wv_ = wqo[:].rearrange("p (c n) -> p c n", c=NC_H)
                                for tkc in range(4):
                                    ps = qps.tile([128, 512], F32, tag="qkps")
                                    for j in range(NC_H // 2):
                                        nc.tensor.matmul(
                                            ps[:],
                                            wv_[:, 2 * j:2 * j + 2, :],
                                            Xv[:, 2 * j:2 * j + 2,
                                               tkc * 512:(tkc + 1) * 512],
                                            start=(j == 0), stop=(j == NC_H // 2 - 1),
                                            perf_mode=DR)
                                    if qk == 0 or lidx == 1:
                                        dst = (Qt if qk == 0 else Kt)[
                                            :, oc * NTOK + tkc * 512:
                                               oc * NTOK + (tkc + 1) * 512]
                                    else:
                                        sp0 = tkc * 8
                                        dst = ktv[:, oc:oc + 1, sp0:sp0 + 8, 0:SPAN]
                                    use_dve = (oc % 2 == 1)
                                    if bias0 and use_dve:
                                        nc.vector.tensor_scalar_mul(dst, ps[:], QCP)
                                    elif bias0:
                                        nc.scalar.activation(dst, ps[:], AF.Copy,
                                                             scale=QCP)
                                    else:
                                        nc.scalar.activation(dst, ps[:], AF.Identity,
                                                             bias=bcol[:, oc:oc + 1],
                                                             scale=QCP)
                        if lidx == 2 and KPHASE >= g_v:
                            # tag K column
                            nc.gpsimd.tensor_copy(
                                ktv[:, :, :, SPAN:SKEY2],
                                tgKs[:].rearrange("p (c s) -> p c s", c=NC_H))
                        # V per span (tokens on psum partitions 0-63)
                        wv = wpool.tile([128, NC_H * H], FP8, tag="wv")
                        nc.sync.dma_start(wv[:], wqkvT[:, 12 * H: 18 * H])
                        wvv = wv[:].rearrange("p (c n) -> p c n", c=NC_H)
                        if lidx == 2 and KPHASE >= g_v:
                            # tag V row: slow single-partition DMA — run it on the
                            # Activation HWDGE queue so it overlaps the SP-queue
                            # weight streams, chunked per span block
                            for tb8 in range(8):
                                w8_ = (NSP // 8) * H
                                nc.sync.dma_start(
                                    Vn[SPAN:SKEY2, tb8 * w8_:(tb8 + 1) * w8_],
                                    tgVd[0:1, tb8 * w8_:(tb8 + 1) * w8_])
                        for sp in range(NSP if KPHASE >= g_v else 0):
                            ps = vqps.tile([64, H], F32, tag="vps")
                            for no in range(2):
                                nw = 512 if no == 0 else 256
                                for j in range(NC_H // 2):
                                    nc.tensor.matmul(
                                        ps[:, no * 512: no * 512 + nw],
                                        Xv[:, 2 * j:2 * j + 2,
                                           sp * SPAN:(sp + 1) * SPAN],
                                        wvv[:, 2 * j:2 * j + 2,
                                            no * 512:no * 512 + nw],
                                        start=(j == 0), stop=(j == NC_H // 2 - 1),
                                        perf_mode=DR)
                            if bias0:
                                if sp % 2 == 0:
                                    nc.scalar.activation(
                                        Vn[0:SPAN, sp * H:(sp + 1) * H], ps[:],
                                        AF.Copy, scale=QCP)
                                else:
                                    nc.vector.tensor_scalar_mul(
                                        Vn[0:SPAN, sp * H:(sp + 1) * H], ps[:], QCP)
                            else:
                                nc.vector.scalar_tensor_tensor(
                                    out=Vn[0:SPAN, sp * H:(sp + 1) * H],
                                    in0=ps[:], scalar=QCP, op0=OP.mult,
                                    in1=bv_bc[0:SPAN, :], op1=OP.add)

                    ret = big24("res2") if lidx == 2 else None
                    with tc.tile_pool(name=f"a{lidx}sb", bufs=3) as asb, \
                         tc.tile_pool(name=f"a{lidx}ps", bufs=1, space="PSUM") as sps, \
                         tc.tile_pool(name=f"a{lidx}dn", bufs=1, space="PSUM") as dps, \
                         tc.tile_pool(name=f"a{lidx}rt", bufs=1, space="PSUM") as rps, \
                         tc.tile_pool(name=f"a{lidx}av", bufs=2, space="PSUM") as avps, \
                         tc.tile_pool(name=f"a{lidx}op", bufs=1, space="PSUM") as opps:
                        # stream out-proj weights once (fp8: 4.5KB/part)
                        wos = awstr.tile([128, NC_H * H], FP8, tag="wos", bufs=1)
                        nc.sync.dma_start(wos[:], woT[:])
                        wosv = wos[:].rearrange("p (c n) -> p c n", c=NC_H)

                        # --- software-pipelined span loop: stage1 (scores+
                        # exp) runs 2 spans ahead of stage3 (recT+normalize)
                        # so the in-order PE queue never waits on the
                        # exp->den->reciprocal->broadcast chain ---
                        st = {}
                        aobs = {}

                        def stage1(s):
                            c0 = s * SPAN
                            scE = sps.tile([KTOT, 384], F32, tag="scE")
                            scO = sps.tile([KTOT, 384], F32, tag="scO")
                            for h in range(NH):
                                hb = (h % 2) * 64
                                dst = scE if h % 2 == 0 else scO
                                if lidx == 1:
                                    kap = Kt[hb:hb + 64,
                                             (h // 2) * NTOK + c0:
                                             (h // 2) * NTOK + c0 + SPAN]
                                else:
                                    kap = ktv[hb:hb + 64, h // 2:h // 2 + 1,
                                              s:s + 1, :]
                                nc.tensor.matmul(
                                    dst[:, (h // 2) * 64:(h // 2 + 1) * 64],
                                    kap,
                                    Qt[hb:hb + 64, (h // 2) * NTOK + c0:
                                       (h // 2) * NTOK + c0 + SPAN],
                                    start=True, stop=True)
                            pexpE = asb.tile([KTOT, 384], BF16, tag="pexpE")
                            pexpO = asb.tile([KTOT, 384], BF16, tag="pexpO")
                            nc.scalar.activation(pexpE[:], scE[:], AF.Exp)
                            nc.scalar.activation(pexpO[:], scO[:], AF.Exp)
                            st[s] = [(pexpE, pexpO)]

                        def stage2(s):
                            pexpE, pexpO = st[s][0]
                            # av_t packs: [:,0:384] attn@V; [0:64,384:396] den;
                            # [0:12,400:464] transposed reciprocal
                            av_t = avps.tile([128, 384], F32, tag="av", bufs=2)
                            den_t = dps.tile([64, NH], F32, tag="den_t")
                            for h in range(NH):
                                pex = pexpE if h % 2 == 0 else pexpO
                                oc = (h // 2) * 64
                                nc.tensor.matmul(
                                    den_t[0:64, h:h + 1],
                                    pex[0:KTOT, oc:oc + 64],
                                    ones_col_bf[0:KTOT, :],
                                    start=True, stop=True)
                            rec_t = asb.tile([64, NH], F32, tag="rec_t")
                            nc.vector.reciprocal_approx_fast(
                                out=rec_t[:], in_=den_t[:])
                            for h in range(NH):
                                hb = (h % 2) * 64
                                oc = (h // 2) * 64
                                pex = pexpE if h % 2 == 0 else pexpO
                                nc.tensor.matmul(
                                    av_t[hb:hb + 64, oc:oc + 64],
                                    Vn[0:KTOT, s * H + h * 64:
                                       s * H + (h + 1) * 64],
                                    pex[0:KTOT, oc:oc + 64],
                                    start=True, stop=True)
                            rec_tb = asb.tile([64, NH], BF16, tag="rec_tb")
                            nc.gpsimd.tensor_copy(rec_tb[:], rec_t[:])
                            rhp = dps.tile([NH, 64], BF16, tag="rhp")
                            nc.tensor.transpose(rhp[:], rec_tb[:],
                                                ident_bf[0:64, 0:64])
                            rec_h = asb.tile([NH, 64], BF16, tag="rec_h")
                            nc.vector.tensor_copy(rec_h[:], rhp[:])
                            st[s] = [av_t, rec_h]

                        def stage3(s):
                            av_t, rec_h = st.pop(s)
                            blk = s // SPB
                            spi = s % SPB
                            if spi == 0:
                                aob = asb.tile([128, NC_H * TBLK], FP8,
                                               tag="aob")
                                aobs[blk] = (aob,
                                             aob[:].rearrange(
                                                 "p (c n) -> p c n", c=NC_H))
                            aobv = aobs[blk][1]
                            # recT[p, c*64+q] = AS / den(head(c, p), q)
                            recT = rps.tile([128, 384], F32, tag="recT",
                                            bufs=1)
                            for c in range(NC_H):
                                nc.tensor.matmul(
                                    recT[:, c * 64:(c + 1) * 64],
                                    asel[:, c * 128:(c + 1) * 128],
                                    rec_h[:, :],
                                    start=True, stop=True)
                            recTs = asb.tile([128, 384], BF16, tag="recTs")
                            if s % 2 == 0:
                                nc.scalar.activation(recTs[:], recT[:],
                                                     AF.Copy)
                            else:
                                nc.vector.tensor_copy(recTs[:], recT[:])
                            nc.vector.tensor_tensor(
                                out=aobv[:, :, spi * SPAN:(spi + 1) * SPAN],
                                in0=av_t[:].rearrange(
                                    "p (c n) -> p c n", c=NC_H),
                                in1=recTs[:].rearrange(
                                    "p (c n) -> p c n", c=NC_H),
                                op=OP.mult)

                        def outproj(blk):
                            aobv = aobs.pop(blk)[1]
                            tb = blk * TBLK
                            if lidx == 1:
                                for oc in range(NC_H):
                                    ps = opps.tile([128, TBLK], F32,
                                                   tag="opps")
                                    if not bias0:
                                        nc.tensor.matmul(
                                            ps[:],
                                            bo1row[:, oc * 128:(oc + 1) * 128],
                                            ones_tb[:, :], start=True,
                                            stop=False)
                                    for j in range(NC_H // 2):
                                        nc.tensor.matmul(
                                            ps[:],
                                            wosv[:, 2 * j:2 * j + 2,
                                                 oc * 128:(oc + 1) * 128],
                                            aobv[:, 2 * j:2 * j + 2, :],
                                            start=(j == 0 and bias0),
                                            stop=(j == NC_H // 2 - 1),
                                            perf_mode=DR)
                                    # X1q = AS*X1 = ps/WS + Xbf  (fp8)
                                    nc.vector.scalar_tensor_tensor(
                                        out=X1q[:, oc * NTOK + tb:
                                                oc * NTOK + tb + TBLK],
                                        in0=ps[:], scalar=1.0 / WS,
                                        op0=OP.mult,
                                        in1=Xbf[:, oc * NTOK + tb:
                                                oc * NTOK + tb + TBLK],
                                        op1=OP.add)
                            else:
                                for tof in range(TBLK // 128):
                                    t = blk * (TBLK // 128) + tof
                                    for no in range(2):
                                        nw = 512 if no == 0 else 256
                                        ps = opps.tile([128, 512], F32,
                                                       tag="opps")
                                        for j in range(NC_H // 2):
                                            nc.tensor.matmul(
                                                ps[:, :nw],
                                                aobv[:, 2 * j:2 * j + 2,
                                                     tof * 128:(tof + 1) * 128],
                                                wosv[:, 2 * j:2 * j + 2,
                                                     no * 512:no * 512 + nw],
                                                start=(j == 0),
                                                stop=(j == NC_H // 2 - 1),
                                                perf_mode=DR)
                                        if bias0:
                                            nc.scalar.activation(
                                                ret[:, t * H + no * 512:
                                                    t * H + no * 512 + nw],
                                                ps[:, :nw], AF.Copy,
                                                scale=QCP * (TAG_RATE if fusedp4
                                                             else 1.0))
                                        else:
                                            nc.vector.scalar_tensor_tensor(
                                                out=ret[:, t * H + no * 512:
                                                        t * H + no * 512 + nw],
                                                in0=ps[:, :nw], scalar=QCP,
                                                op0=OP.mult,
                                                in1=bo2_bc[:,
                                                           no * 512:
                                                           no * 512 + nw],
                                                op1=OP.add)
                            if post_blk is not None:
                                post_blk(blk, ret)

                        if KPHASE >= g_at:
                            for it in range(NSP + 2):
                                if it < NSP:
                                    stage1(it)
                                if 1 <= it <= NSP:
                                    stage2(it - 1)
                                if 2 <= it <= NSP + 1:
                                    s3 = it - 2
                                    stage3(s3)
                                    if s3 % SPB == SPB - 1:
                                        outproj(s3 // SPB)
                    return ret

                # ============ Phase 2+3: the two attention layers ============
                X1q = big12("X1q")
                res2 = None
                fusedp4 = identg and an_trivial and KPHASE >= 10

                with tc.tile_pool(name="wqs", bufs=6) as wstr_s, \
                     tc.tile_pool(name="wvs", bufs=1) as wpool_s, \
                     tc.tile_pool(name="wos_s", bufs=1) as awstr_s:
                    wpools = (wstr_s, wpool_s, awstr_s)
                    with tc.tile_pool(name="att1", bufs=1) as lp1:
                        attention(1, Xt8, bq1, bk1, wqkv1q, woq1, bv1_bc, X1q,
                                  None, lp1, wpools)
                    xbf_cm.__exit__(None, None, None)
                    if KPHASE >= 7:
                        with tc.tile_pool(name="att2", bufs=1) as lp2, \
                             tc.tile_pool(name="tgkp", bufs=1) as tgkp:
                            tgKs = tgkp.tile([128, NC_H * NSP], BF16, tag="tgKs")
                            nc.sync.dma_start(
                                tgKs[:].rearrange("p (c n) -> p c n", c=NC_H),
                                tgKd[:].rearrange("(c p) n -> p c n", p=128))
                            res2 = attention(2, X1q, bq2, bk2, wqkv2q, woq2,
                                             bv2_bc, None, tgKs, lp2, wpools)
                pbigB_cm = tc.tile_pool(name="pbigB", bufs=1)
                pbigB = pbigB_cm.__enter__()
                out_emb = pbigB.tile([128, NT16 * H], BF16, tag="e24",
                                     bufs=1, name="out_emb")

                # ============ Phase 4 (fallback): scatter + LN1 ============
                with tc.tile_pool(name="p4sb", bufs=3) as p4, \
                     tc.tile_pool(name="p4ln", bufs=2) as p4ln, \
                     tc.tile_pool(name="p4ps", bufs=2, space="PSUM") as p4ps:
                    moff = 0
                    for c in range(0 if fusedp4 else
                                   (NT16 if KPHASE >= 10 else 0)):
                        raw = p4.tile([128, H], F32, tag="raw")
                        nc.sync.dma_start(raw[:], we[c * 128:(c + 1) * 128, :])
                        comb = p4.tile([128, H], F32, tag="comb")
                        ics = kranges[c]
                        if identg:
                            # spans tile the sequence: res2 rows ARE tokens
                            nc.vector.scalar_tensor_tensor(
                                out=comb[:, 0:512], in0=res2[:, c * H:c * H + 512],
                                scalar=TAG_RATE, in1=raw[:, 0:512],
                                op0=OP.mult, op1=OP.add)
                            nc.vector.scalar_tensor_tensor(
                                out=comb[:, 512:768],
                                in0=res2[:, c * H + 512:(c + 1) * H],
                                scalar=TAG_RATE, in1=raw[:, 512:768],
                                op0=OP.mult, op1=OP.add)
                        elif len(ics) > 0:
                            tps = p4ps.tile([128, 512], F32, tag="tps")
                            tps2 = p4ps.tile([128, 256], F32, tag="tps2")
                            for j, ic in enumerate(ics):
                                mb = p4.tile([128, 128], BF16, tag="mb")
                                nc.sync.dma_start(
                                    mb[:], mmatc[moff * 128:(moff + 1) * 128, :])
                                moff += 1
                                nc.tensor.matmul(tps[:], mb[:],
                                                 res2[:, ic * H: ic * H + 512],
                                                 start=(j == 0), stop=(j == len(ics) - 1))
                                nc.tensor.matmul(tps2[:], mb[:],
                                                 res2[:, ic * H + 512: (ic + 1) * H],
                                                 start=(j == 0), stop=(j == len(ics) - 1))
                            nc.vector.scalar_tensor_tensor(
                                out=comb[:, 0:512], in0=tps[:], scalar=TAG_RATE,
                                in1=raw[:, 0:512], op0=OP.mult, op1=OP.add)
                            nc.vector.scalar_tensor_tensor(
                                out=comb[:, 512:768], in0=tps2[:], scalar=TAG_RATE,
                                in1=raw[:, 512:768], op0=OP.mult, op1=OP.add)
                        else:
                            nc.gpsimd.tensor_copy(comb[:], raw[:])
                        _ln_apply(nc, p4ln, comb, slice(None), "ln1", ang_bc, anb_bc,
                                  an_trivial, eps_t, out_emb[:, c * H:(c + 1) * H])

                if f1:
                    oeT = big12("oeT8")
                    oescale = AS
                elif fusedp4:
                    # res2 is still live while oeT fills (interleaved with FFN)
                    oeT = pbigB.tile([128, NC_H * NTOK], BF16, tag="oet",
                                     bufs=1, name="oeT")
                    oescale = 1.0
                else:
                    oeT = big24("oeT")
                    oescale = 1.0
                oevw = oeT[:].rearrange("p (c n) -> p c n", c=NC_H)

                def oet_transpose(t, psp):
                    """transpose LN1-out tile t into oeT via PE (bf16)."""
                    tpT = psp.tile([128, H], BF16, tag="tpT", bufs=2)
                    for c in range(NC_H):
                        nc.tensor.transpose(
                            tpT[:, c * 128:(c + 1) * 128],
                            out_emb[:, t * H + c * 128: t * H + (c + 1) * 128],
                            ident_bf[:])
                    nc.scalar.activation(
                        oevw[:, 0:4, t * 128:(t + 1) * 128],
                        tpT[:, 0:512].rearrange("p (c n) -> p c n", c=4),
                        AF.Copy, scale=oescale)
                    if oescale == 1.0:
                        nc.vector.tensor_copy(
                            oevw[:, 4:6, t * 128:(t + 1) * 128],
                            tpT[:, 512:768].rearrange("p (c n) -> p c n", c=2))
                    else:
                        nc.vector.tensor_scalar_mul(
                            oevw[:, 4:6, t * 128:(t + 1) * 128],
                            tpT[:, 512:768].rearrange("p (c n) -> p c n", c=2),
                            oescale)

                # transpose out_emb -> oeT (non-fused fallback)
                with tc.tile_pool(name="p5ps2", bufs=2, space="PSUM") as p5ps:
                    for t in range(0 if fusedp4 else
                                   (NT16 if KPHASE >= 11 else 0)):
                        oet_transpose(t, p5ps)
                oev = oeT[:].rearrange("p (c n) -> p c n", c=NC_H)

                # ============ Phase 5: FFN + LN2 (in-place into out_emb) ========
                h1scale = (QCP if f1 else 1.0) * (AS if f2 else 1.0)
                ffscale = QCP if f2 else 1.0
                with tc.tile_pool(name="w5", bufs=1) as w5, \
                     tc.tile_pool(name="w5s", bufs=3) as w5s, \
                     tc.tile_pool(name="p45", bufs=1) as p45, \
                     tc.tile_pool(name="p45ln", bufs=2) as p45ln, \
                     tc.tile_pool(name="ffln", bufs=2) as ffln, \
                     tc.tile_pool(name="ffps", bufs=2, space="PSUM") as ffps, \
                     tc.tile_pool(name="ffps2", bufs=2, space="PSUM") as ffps2, \
                     tc.tile_pool(name="p6ps", bufs=1, space="PSUM") as p6ps, \
                     tc.tile_pool(name="p7ps", bufs=1, space="PSUM") as p7ps, \
                     tc.tile_pool(name="p7sb", bufs=2) as p7sb:
                    if KPHASE >= 14:
                        wout = w5.tile([128, NC_H * NL], BF16, tag="wout")
                        nc.sync.dma_start(
                            wout[:].rearrange("p (c n) -> p c n", c=NC_H),
                            woutT[:].rearrange("(c p) n -> p c n", p=128))
                    raw_tiles = {}

                    def prefetch_raw(cs):
                        for c in cs:
                            r = p45.tile([128, H], BF16, tag="raw",
                                         bufs=8, name=f"raw{c}")
                            nc.sync.dma_start(
                                r[:], web[c * 128:(c + 1) * 128, :])
                            raw_tiles[c] = r

                    def phase4_tile(c):
                        # merge scatter result + residual, LN1 in place;
                        # alternate DVE / Pool so the boundary drains fast
                        raw = raw_tiles.pop(c)
                        oe = out_emb[:, c * H:(c + 1) * H]
                        eng = nc.vector if c % 2 == 0 else nc.gpsimd
                        eng.tensor_tensor(
                            out=oe, in0=res2[:, c * H:(c + 1) * H],
                            in1=raw[:], op=OP.add)
                        _ln_fast(nc, p45ln, oe, "ln1", eps_t, oe,
                                 apply_eng=(nc.gpsimd if c % 2 == 0
                                            else nc.vector))

                    if fusedp4:
                        prefetch_raw(range(0, 8))
                    w2t = w5.tile([128, NC_I * H], W2DT, tag="w2t")
                    nc.sync.dma_start(w2t[:], w2d[:])
                    w2v = w2t[:].rearrange("p (c n) -> p c n", c=NC_I)
                    for tp in range(2 if KPHASE >= 12 else 0):
                        if fusedp4 and tp == 0:
                            prefetch_raw(range(8, 16))
                        h1s = [pbigB.tile([128, NC_I * 512], H1DT, tag="h1",
                                          bufs=2, name=f"h1_{2 * tp + i}")
                               for i in range(2)]

                        def stage1_pass(tki, tp=tp, h1s=h1s):
                            # one tkc per pass: W1 is streamed twice, but the
                            # PE can start on tkc0 while phase-4 of the other
                            # 4 tiles is still draining
                            tkc = 2 * tp + tki
                            h1 = h1s[tki]
                            for fc in range(NC_I):
                                w1s = w5s.tile([128, NC_H * 128], W1DT,
                                               tag="w1s")
                                nc.sync.dma_start(
                                    w1s[:], w1d[:, fc * H:(fc + 1) * H])
                                w1v = w1s[:].rearrange("p (c n) -> p c n",
                                                       c=NC_H)
                                ps = ffps.tile([128, 512], F32, tag="h1ps")
                                if f1:
                                    for j in range(NC_H // 2):
                                        nc.tensor.matmul(
                                            ps[:], w1v[:, 2 * j:2 * j + 2, :],
                                            oev[:, 2 * j:2 * j + 2,
                                                tkc * 512:(tkc + 1) * 512],
                                            start=(j == 0),
                                            stop=(j == NC_H // 2 - 1),
                                            perf_mode=DR)
                                else:
                                    for ic in range(NC_H):
                                        nc.tensor.matmul(
                                            ps[:],
                                            w1s[:, ic * 128:(ic + 1) * 128],
                                            oeT[:, ic * NTOK + tkc * 512:
                                                ic * NTOK + (tkc + 1) * 512],
                                            start=(ic == 0),
                                            stop=(ic == NC_H - 1))
                                if bias0:
                                    if fc % 2 == 0:
                                        nc.vector.tensor_scalar(
                                            out=h1[:, fc * 512:(fc + 1) * 512],
                                            in0=ps[:], scalar1=h1scale,
                                            scalar2=0.0,
                                            op0=OP.mult, op1=OP.max)
                                    else:
                                        nc.scalar.activation(
                                            h1[:, fc * 512:(fc + 1) * 512],
                                            ps[:], AF.Relu, scale=h1scale)
                                else:
                                    nc.scalar.activation(
                                        h1[:, fc * 512:(fc + 1) * 512],
                                        ps[:], AF.Relu, bias=b1t[:, fc:fc + 1],
                                        scale=h1scale)

                        if fusedp4:
                            for c in range(tp * 8, tp * 8 + 4):
                                phase4_tile(c)
                            for t in range(tp * 8, tp * 8 + 4):
                                oet_transpose(t, p6ps)
                            stage1_pass(0)
                            for c in range(tp * 8 + 4, tp * 8 + 8):
                                phase4_tile(c)
                            for t in range(tp * 8 + 4, tp * 8 + 8):
                                oet_transpose(t, p6ps)
                            stage1_pass(1)
                        else:
                            stage1_pass(0)
                            stage1_pass(1)
                        for tt8 in range(8):
                            tkc = 2 * tp + tt8 // 4
                            tt = tt8 % 4
                            h1 = h1s[tt8 // 4]
                            h1v = h1[:].rearrange("p (c n) -> p c n", c=NC_I)
                            t = tkc * 4 + tt
                            for no in range(2):
                                nw = 512 if no == 0 else 256
                                ps2 = ffps2.tile([128, 512], F32, tag="ffout")
                                if not bias0:
                                    # b2 via ones x b2row init
                                    nc.tensor.matmul(
                                        ps2[:, :nw], ones_row_bf[:, :],
                                        b2row[:, no * 512:no * 512 + nw],
                                        start=True, stop=False)
                                if f2:
                                    for j in range(NC_I // 2):
                                        nc.tensor.matmul(
                                            ps2[:, :nw],
                                            h1v[:, 2 * j:2 * j + 2,
                                                tt * 128:(tt + 1) * 128],
                                            w2v[:, 2 * j:2 * j + 2,
                                                no * 512:no * 512 + nw],
                                            start=(j == 0 and bias0),
                                            stop=(j == NC_I // 2 - 1),
                                            perf_mode=DR)
                                else:
                                    for fc in range(NC_I):
                                        nc.tensor.matmul(
                                            ps2[:, :nw],
                                            h1[:, fc * 512 + tt * 128:
                                               fc * 512 + (tt + 1) * 128],
                                            w2t[:, fc * H + no * 512:
                                                fc * H + no * 512 + nw],
                                            start=(fc == 0 and bias0),
                                            stop=(fc == NC_I - 1))
                                nc.vector.scalar_tensor_tensor(
                                    out=out_emb[:, t * H + no * 512:
                                                t * H + no * 512 + nw],
                                    in0=ps2[:, :nw], scalar=ffscale, op0=OP.mult,
                                    in1=out_emb[:, t * H + no * 512:
                                                t * H + no * 512 + nw],
                                    op1=OP.add)
                            if fn_trivial:
                                _ln_fast(nc, ffln, out_emb[:, t * H:(t + 1) * H],
                                         "ln2", eps_t,
                                         out_emb[:, t * H:(t + 1) * H])
                            else:
                                _ln_apply(nc, ffln, out_emb,
                                          slice(t * H, (t + 1) * H), "ln2",
                                          fng_bc, fnb_bc, fn_trivial, eps_t,
                                          out_emb[:, t * H:(t + 1) * H])
                            if KPHASE < 13:
                                continue
                            # fused tail: transpose ln2(t) -> ln2T, Wout matmul,
                            # store — overlaps the old post-FFN tail into the
                            # PE-busy FFN window
                            tpT = p6ps.tile([128, H], BF16, tag="tpT", bufs=2)
                            for c in range(NC_H):
                                nc.tensor.transpose(
                                    tpT[:, c * 128:(c + 1) * 128],
                                    out_emb[:, t * H + c * 128: t * H + (c + 1) * 128],
                                    ident_bf[:])
                            lt = p7sb.tile([128, H], BF16, tag="l2t", bufs=3)
                            nc.scalar.activation(lt[:, 0:512], tpT[:, 0:512],
                                                 AF.Copy)
                            nc.vector.tensor_copy(lt[:, 512:768], tpT[:, 512:768])
                            if KPHASE >= 14:
                                ps3 = p7ps.tile([128, NL], F32, tag="outps")
                                for ic in range(NC_H):
                                    nc.tensor.matmul(
                                        ps3[:],
                                        lt[:, ic * 128:(ic + 1) * 128],
                                        wout[:, ic * NL:(ic + 1) * NL],
                                        start=(ic == 0), stop=(ic == NC_H - 1))
                                ot = p7sb.tile([128, NL], F32, tag="ot")
                                if bias0:
                                    nc.vector.tensor_copy(ot[:], ps3[:])
                                else:
                                    nc.vector.tensor_tensor(
                                        out=ot[:], in0=ps3[:],
                                        in1=bout_bc[:, :NL], op=OP.add)
                                nc.sync.dma_start(out_d[t * 128:(t + 1) * 128, :],
                                                  ot[:])
                    if KPHASE < 14:
                        for t in range(NT16):
                            nc.sync.dma_start(out_d[t * 128:(t + 1) * 128, :],
                                              we[t * 128:(t + 1) * 128, 0:NL])
                pbigB_cm.__exit__(None, None, None)

    return nc


def _host_prep(word_embedding, span_starts, span_tags, tag_emb,
               sa_Wqkv, sa_bqkv, sa_Wo, sa_bo,
               at_Wqkv, at_bqkv, at_Wo, at_bo,
               an_g, an_b, W1, b1, W2, b2, fn_g, fn_b, Wout, bout):
    bf = ml_dtypes.bfloat16
    f8 = ml_dtypes.float8_e4m3
    we = np.ascontiguousarray(np.asarray(word_embedding, dtype=np.float32))
    starts = np.asarray(span_starts).astype(np.int64)
    tags = np.asarray(span_tags).astype(np.int64)
    pos = (starts[:, :, None] + np.arange(SPAN)[None, None, :]).reshape(B, NTOK)

    half = H // 2
    inv = 1.0 / (10000.0 ** (np.arange(half, dtype=np.float64) / half))
    ang = np.arange(S, dtype=np.float64)[:, None] * inv[None, :]
    cos_t = np.cos(ang).astype(np.float32)
    sin_t = np.sin(ang).astype(np.float32)

    # scatter selection blocks + union k-ranges
    kr_lo = np.full(NT16, NTOK, dtype=np.int64)
    kr_hi = np.zeros(NT16, dtype=np.int64)
    for b in range(B):
        for c in range(NT16):
            js = np.nonzero((starts[b] < (c + 1) * 128) & (starts[b] + SPAN > c * 128))[0]
            if len(js) > 0:
                kr_lo[c] = min(kr_lo[c], js[0] * SPAN)
                kr_hi[c] = max(kr_hi[c], (js[-1] + 1) * SPAN)
    kranges = []
    for c in range(NT16):
        if kr_hi[c] <= kr_lo[c]:
            kranges.append(())
        else:
            kranges.append(tuple(range(int(kr_lo[c]) // 128,
                                       int(np.ceil(kr_hi[c] / 128)))))
    kranges = tuple(kranges)

    f32 = lambda x: np.ascontiguousarray(np.asarray(x, dtype=np.float32))
    tobf = lambda x: np.ascontiguousarray(np.asarray(x, dtype=np.float32).astype(bf))
    tof8 = lambda x: np.ascontiguousarray(np.asarray(x, dtype=np.float32).astype(f8))
    sa_Wqkv, sa_bqkv, sa_Wo, sa_bo = f32(sa_Wqkv), f32(sa_bqkv), f32(sa_Wo), f32(sa_bo)
    at_Wqkv, at_bqkv, at_Wo, at_bo = f32(at_Wqkv), f32(at_bqkv), f32(at_Wo), f32(at_bo)
    an_g, an_b, fn_g, fn_b = f32(an_g), f32(an_b), f32(fn_g), f32(fn_b)
    W1, b1, W2, b2, Wout, bout = f32(W1), f32(b1), f32(W2), f32(b2), f32(Wout), f32(bout)
    tag_emb = f32(tag_emb)
    f1 = KFFN in ("fp8", "mix1")
    f2 = KFFN in ("fp8", "mix2")

    def qkv_q(Wqkv):
        # [3H, H] -> [128, (qk, oc, c, n) | (c, n768)] fp8: Q section
        # pre-scaled by SCALE*WS, K/V by WS, pre-layouted so each kernel
        # stream DMA is contiguous on both sides
        wT = Wqkv.T.copy()
        wT[:, 0:H] *= SCALE * WS
        wT[:, H:] *= WS
        qk = (wT[:, :2 * H].reshape(NC_H, 128, 2, NC_H, 128)
              .transpose(1, 2, 3, 0, 4).reshape(128, -1))
        v = (wT[:, 2 * H:].reshape(NC_H, 128, H)
             .transpose(1, 0, 2).reshape(128, -1))
        return tof8(np.concatenate([qk, v], axis=1))

    def wo_q(Wo):
        return tof8((Wo.T * WS).reshape(NC_H, 128, H)
                    .transpose(1, 0, 2).reshape(128, -1))

    shared = {
        "wq1p": qkv_q(sa_Wqkv), "wo1p": wo_q(sa_Wo),
        "wq2p": qkv_q(at_Wqkv), "wo2p": wo_q(at_Wo),
        "woutT": tobf(Wout.T),
        "bq1c": np.ascontiguousarray((sa_bqkv[0:H] * SCALE).reshape(NC_H, 128).T),
        "bk1c": np.ascontiguousarray(sa_bqkv[H:2 * H].reshape(NC_H, 128).T),
        "bq2c": np.ascontiguousarray((at_bqkv[0:H] * SCALE).reshape(NC_H, 128).T),
        "bk2c": np.ascontiguousarray(at_bqkv[H:2 * H].reshape(NC_H, 128).T),
        "vec_bo1r": (sa_bo * AS * WS).reshape(1, H),
        "b1c": np.ascontiguousarray(
            (b1 * (AS if f2 else 1.0)).reshape(NC_I, 128).T),
        "vec_bv1": sa_bqkv[2 * H:].reshape(1, H),
        "vec_bv2": at_bqkv[2 * H:].reshape(1, H),
        "vec_bo2": at_bo.reshape(1, H),
        "vec_b2": (b2 * (AS * WS if f2 else 1.0)).reshape(1, H),
        "vec_ang": an_g.reshape(1, H), "vec_anb": an_b.reshape(1, H),
        "vec_fng": fn_g.reshape(1, H), "vec_fnb": fn_b.reshape(1, H),
        "vec_bout": bout.reshape(1, NL),
    }
    w1p = (W1.T * (WS if f1 else 1.0)).reshape(NC_H, 128, NC_I, 128) \
        .transpose(1, 2, 0, 3).reshape(128, -1)
    w2p = (W2.T * (WS if f2 else 1.0)).reshape(NC_I, 128, H) \
        .transpose(1, 0, 2).reshape(128, -1)
    shared["w1pp"] = tof8(w1p) if f1 else tobf(w1p)
    shared["w2pp"] = tof8(w2p) if f2 else tobf(w2p)
    an_trivial = bool(np.all(an_g == 1.0) and np.all(an_b == 0.0))
    fn_trivial = bool(np.all(fn_g == 1.0) and np.all(fn_b == 0.0))
    bias0 = bool(all(np.all(v == 0.0) for v in
                     (sa_bqkv, sa_bo, at_bqkv, at_bo, b1, b2, bout)))
    identg = bool(np.all(pos == np.arange(NTOK)[None, :]))

    at_Wk = at_Wqkv[H:2 * H]
    at_Wv = at_Wqkv[2 * H:]
    bk2 = at_bqkv[H:2 * H]
    bv2 = at_bqkv[2 * H:]

    in_maps = []
    for b in range(B):
        pb = pos[b]
        tagv = tag_emb[tags[b]]  # [NSP, H]
        tagK = tagv @ at_Wk.T + bk2
        tagV = tagv @ at_Wv.T + bv2
        blocks = []
        for c in range(NT16):
            for ic in kranges[c]:
                m = (pb[ic * 128:(ic + 1) * 128, None]
                     == np.arange(c * 128, (c + 1) * 128)[None, :])
                blocks.append(m.astype(bf))
        mmatc = (np.concatenate(blocks, axis=0) if blocks
                 else np.zeros((128, 128), bf))
        m = {
            "we": we[b],
            "web": np.ascontiguousarray(we[b].astype(bf)),
            "posi": np.ascontiguousarray(pb.reshape(NT16, 128).T.astype(np.int32)),
            "cosg": np.ascontiguousarray((AS * cos_t[pb]).astype(bf)),
            "sing": np.ascontiguousarray((AS * sin_t[pb]).astype(bf)),
            "mmatc": np.ascontiguousarray(mmatc),
            "tgKd": np.ascontiguousarray(tagK.T.astype(bf)),
            "tgVd": np.ascontiguousarray(tagV.reshape(1, NSP * H).astype(bf)),
        }
        m.update(shared)
        in_maps.append(m)
    return in_maps, kranges, an_trivial, fn_trivial, bias0, identg


def kernel(**inputs):
    in_maps, kranges, an_trivial, fn_trivial, bias0, identg = _host_prep(**inputs)
    key = (kranges, an_trivial, fn_trivial, bias0, identg, KFFN)
    if key not in _CACHE:
        _CACHE[key] = build(kranges, an_trivial, fn_trivial, bias0, identg)
    nc = _CACHE[key]
    res = run_bass_kernel_spmd(nc, in_maps, core_ids=list(range(8)), trace=False)
    out = np.stack([res.results[b]["out"] for b in range(B)], axis=0)
    return out.astype(np.float32)


if __name__ == "__main__":
    import reference
    inputs = {k: np.asarray(v) for k, v in reference.setup_inputs().items()}
    exp = np.asarray(reference.reference(**inputs))
    got = kernel(**inputs)
    err = np.abs(got - exp).max()
    rel = err / np.abs(exp).max()
    print("max abs err:", err, "rel:", rel)

